# revision 3
# baseline (speedup 1.0000x reference)
"""GAT-KH (2-layer, 3-hop, 8-head GAT, N=50k, E=300k/hop) on 8 TRN2 cores.

Distribution: dst-sharded edges, replicated node-side tables.
- Nodes renumbered into 8 padded shards of 6400 (NP=51200). Core c owns dst
  shard c (tiles of 128 dsts, 50 tiles).
- Per (layer,hop): gather table T_k[n] = [hp(256) | alpha_src(8) |
  alpha_dst(8)] bf16, 768B row stride, computed replicated on every core
  from transposed h and host-folded Wcat = [W | W@a_src | W@a_dst].
- Edge phase: per dst tile, edges chunked by 128 (split src<32768 for int16
  dma_gather), gathered rows -> softmax numerators w=exp(leaky(as+ad)+mb)
  -> one-hot P matmuls scatter [w*hp | w] into PSUM -> normalize -> dec_w
  -> leaky -> decay-accumulate.
- Layer end: LayerNorm+residual; layer-1 h shards AllGathered for layer-2
  tables. Output: per-core h shard, host-concatenated.
"""

import numpy as np
import ml_dtypes
from contextlib import ExitStack

N = 50000
E = 300000
HOPS = 3
LAYERS = 2
HEADS = 8
D = 256
DH = 32
NCORES = 8
SHARD = 6250
SHARD_P = 6400
TILES = SHARD_P // 128            # 50
NP = NCORES * SHARD_P             # 51200
NT = NP // 128                    # 400 table chunks
GT = 10                           # table chunks per staging group (50%GT==0)
ROWB = 384                        # table row length in bf16 elems (768B)
TCOLS = 272
LH_SPLIT = 32768
DECAY = [float(np.exp(-0.5 * k)) for k in range(HOPS)]
SLOPE_ACT = 0.01
SLOPE_ATT = 0.2
LN_EPS = 1e-5
NEG_BIAS = -30000.0
TPG = 5                           # dst tiles per gather group (50%TPG==0)
BF16 = ml_dtypes.bfloat16


def _pack_idx16(idx):
    """int16 idx list -> [128, ceil(n/16)] wrapped in 16 partitions, x8."""
    n = len(idx)
    n16 = max(1, (n + 15) // 16)
    a = np.zeros((16, n16), np.int16)
    for p in range(16):
        seg = idx[p::16]
        a[p, : len(seg)] = seg
    return np.tile(a, (8, 1))


def _host_prep(inputs):
    x = np.asarray(inputs["x"], np.float32)
    ei = np.asarray(inputs["edge_index_k_hops"])
    lin1_w = np.asarray(inputs["lin1_w"], np.float32)
    gat_w = np.asarray(inputs["gat_w"], np.float32)
    a_src = np.asarray(inputs["gat_att_src"], np.float32)
    a_dst = np.asarray(inputs["gat_att_dst"], np.float32)
    dec_w = np.asarray(inputs["dec_w"], np.float32)

    wcat = np.zeros((LAYERS, HOPS, D, TCOLS), np.float32)
    for l in range(LAYERS):
        for k in range(HOPS):
            W = gat_w[l, k]
            Wh = W.reshape(D, HEADS, DH)
            wcat[l, k, :, :D] = W
            wcat[l, k, :, D:D + HEADS] = np.einsum("dhc,hc->dh", Wh, a_src[l, k])
            wcat[l, k, :, D + HEADS:] = np.einsum("dhc,hc->dh", Wh, a_dst[l, k])

    xT = np.zeros((D, NP), np.float32)
    for s in range(NCORES):
        xs = x[s * SHARD:(s + 1) * SHARD]
        xT[:, s * SHARD_P: s * SHARD_P + xs.shape[0]] = xs.T
    xT_bf = xT.astype(BF16)
    # per-core shard slice of xT, shaped [2,128,SHARD_P]
    xTs = [
        xT_bf[:, c * SHARD_P:(c + 1) * SHARD_P].reshape(2, 128, SHARD_P)
        for c in range(NCORES)
    ]

    # ---- edge routing ----
    hopdat = []
    cnts = np.zeros((HOPS, NCORES, TILES, 2), np.int64)
    for k in range(HOPS):
        src = ei[k, 0].astype(np.int64)
        dst = ei[k, 1].astype(np.int64)
        ps = (src // SHARD) * SHARD_P + (src % SHARD)
        core = dst // SHARD
        dl = dst % SHARD
        tl = dl // 128
        dloc = dl % 128
        low = ps < LH_SPLIT
        hopdat.append((ps, core, tl, dloc, low))
        for c in range(NCORES):
            m = core == c
            tls = tl[m]
            lows = low[m]
            for t in range(TILES):
                mt = tls == t
                cnts[k, c, t, 0] = int((mt & lows).sum())
                cnts[k, c, t, 1] = int((mt & ~lows).sum())
    nch = np.maximum(1, np.ceil(cnts.max(axis=1) / 128.0)).astype(np.int64)  # [HOPS,TILES,2]

    core_data = []
    for c in range(NCORES):
        hops = []
        for k in range(HOPS):
            ps, core, tl, dloc, low = hopdat[k]
            m = core == c
            Lparts, Hparts, dcols, mcols = [], [], [], []
            for t in range(TILES):
                mt = m & (tl == t)
                for side in (0, 1):
                    msk = mt & (low if side == 0 else ~low)
                    n_real = int(msk.sum())
                    cap = int(nch[k, t, side]) * 128
                    idx = np.zeros(cap, np.int64)
                    dv = np.zeros(cap, np.int64)
                    mb = np.full(cap, NEG_BIAS, np.float32)
                    idx[:n_real] = ps[msk] - (LH_SPLIT if side else 0)
                    dv[:n_real] = dloc[msk]
                    mb[:n_real] = 0.0
                    (Lparts if side == 0 else Hparts).append(idx)
                    nc_ = cap // 128
                    dcols.append(dv.reshape(nc_, 128).T.astype(np.float32))
                    mcols.append(mb.reshape(nc_, 128).T.astype(np.float32))
            Lidx = np.concatenate(Lparts).astype(np.int16)
            Hidx = np.concatenate(Hparts).astype(np.int16)
            dstloc = np.concatenate(dcols, axis=1)
            # flat edge-order dst-local row: dstrow[chunk*128+p] = dstloc[p, chunk]
            dstrow = dstloc.T.reshape(1, -1).astype(BF16)
            hops.append({
                "Lidx": _pack_idx16(Lidx), "Hidx": _pack_idx16(Hidx),
                "dstloc": dstloc, "dstrow": dstrow,
                "maskb": np.concatenate(mcols, axis=1),
            })
        core_data.append(hops)

    iota = np.tile(np.arange(128, dtype=np.float32)[None, :], (128, 1))
    iotac = np.ascontiguousarray(np.arange(128, dtype=np.float32)[:, None])
    return {
        "iotac": iotac,
        "wcat": wcat.astype(BF16), "xT": xT_bf, "xTs": xTs,
        "lin1": lin1_w.astype(BF16), "dec": dec_w.astype(BF16),
        "core_data": core_data, "nch": nch, "iota": iota,
    }


def _build(prep, stage=5, edgelvl=5):
    from concourse import bass, mybir, tile, library_config
    from concourse.bass import AP
    from concourse.masks import make_identity
    import concourse.bacc as bacc

    nch = prep["nch"]
    cd0 = prep["core_data"][0]
    NCH = [cd0[k]["dstloc"].shape[1] for k in range(HOPS)]
    NIDXL = [cd0[k]["Lidx"].shape[1] * 16 for k in range(HOPS)]
    NIDXH = [cd0[k]["Hidx"].shape[1] * 16 for k in range(HOPS)]

    fp32 = mybir.dt.float32
    bf16 = mybir.dt.bfloat16
    i16 = mybir.dt.int16
    AF = mybir.ActivationFunctionType
    OP = mybir.AluOpType

    nc = bacc.Bacc("TRN2", target_bir_lowering=False, debug=False,
                   num_devices=NCORES)

    d_xT = nc.dram_tensor("xT", [D, NP], bf16, kind="ExternalInput")
    d_xTs = nc.dram_tensor("xTs", [2, 128, SHARD_P], bf16, kind="ExternalInput")
    d_lin1 = nc.dram_tensor("lin1", [D, D], bf16, kind="ExternalInput")
    d_wcat = nc.dram_tensor("wcat", [LAYERS, HOPS, D, TCOLS], bf16, kind="ExternalInput")
    d_dec = nc.dram_tensor("dec", [LAYERS, HOPS, D, D], bf16, kind="ExternalInput")
    d_iota = nc.dram_tensor("iota", [128, 128], fp32, kind="ExternalInput")
    d_gidx = [(nc.dram_tensor(f"gidxL{k}", [128, NIDXL[k] // 16], i16, kind="ExternalInput"),
               nc.dram_tensor(f"gidxH{k}", [128, NIDXH[k] // 16], i16, kind="ExternalInput"))
              for k in range(HOPS)]
    d_dstloc = [nc.dram_tensor(f"dstloc{k}", [128, NCH[k]], fp32, kind="ExternalInput") for k in range(HOPS)]
    d_dstrow = [nc.dram_tensor(f"dstrow{k}", [1, NCH[k] * 128], bf16, kind="ExternalInput") for k in range(HOPS)]
    d_iotac = nc.dram_tensor("iotac", [128, 1], fp32, kind="ExternalInput")
    d_maskb = [nc.dram_tensor(f"maskb{k}", [128, NCH[k]], fp32, kind="ExternalInput") for k in range(HOPS)]
    d_out = nc.dram_tensor("out", [SHARD_P, D], fp32, kind="ExternalOutput")

    d_tab = [nc.dram_tensor(f"tab{k}", [NP, ROWB], bf16, kind="Internal") for k in range(HOPS)]
    d_hT0 = nc.dram_tensor("hT0", [2, 128, NP], bf16, kind="Internal")
    d_agin = nc.dram_tensor("agin", [2, 128, SHARD_P], bf16, kind="Internal")
    d_agout = nc.dram_tensor("agout", [NCORES, 2, 128, SHARD_P], bf16,
                             kind="Internal", addr_space="Shared")

    # per-hop static chunk bookkeeping
    # chunk columns: per tile t: L chunks then H chunks
    col0 = []   # [k][t] first chunk col of tile t
    lcum = []   # [k][t] first L-chunk index (global, within hop) of tile t
    hcum = []
    for k in range(HOPS):
        c0, lc, hc = [], [], []
        a = b = g = 0
        for t in range(TILES):
            c0.append(g)
            lc.append(a)
            hc.append(b)
            a += int(nch[k, t, 0])
            b += int(nch[k, t, 1])
            g += int(nch[k, t, 0] + nch[k, t, 1])
        col0.append(c0)
        lcum.append(lc)
        hcum.append(hc)

    groups = [list(range(g0, g0 + TPG)) for g0 in range(0, TILES, TPG)]

    def bcast_mid(ap, n):
        """[128, M] AP -> [128, n, M] with 0-step middle dim."""
        return AP(ap.tensor, ap.offset, [list(ap.ap[0]), [0, n], list(ap.ap[1])])

    with tile.TileContext(nc) as tc:
        with ExitStack() as ctx:
            persist = ctx.enter_context(tc.tile_pool(name="persist", bufs=1))
            nc.gpsimd.load_library(library_config.mlp)

            sb_iota = persist.tile((128, 128), fp32)
            nc.sync.dma_start(sb_iota[:], d_iota[:, :])
            sb_ident = persist.tile((128, 128), bf16)
            make_identity(nc, sb_ident[:])
            sb_lin1 = persist.tile((128, 2, D), bf16)
            for kc in range(2):
                nc.sync.dma_start(sb_lin1[:, kc, :], d_lin1[kc * 128:(kc + 1) * 128, :])
            sb_wcat = persist.tile((128, LAYERS, HOPS, 2, TCOLS), bf16)
            sb_dec = persist.tile((128, LAYERS, HOPS, 2, D), bf16)
            for l in range(LAYERS):
                for k in range(HOPS):
                    for kc in range(2):
                        nc.sync.dma_start(sb_wcat[:, l, k, kc, :],
                                          d_wcat[l, k, kc * 128:(kc + 1) * 128, :])
                        nc.sync.dma_start(sb_dec[:, l, k, kc, :],
                                          d_dec[l, k, kc * 128:(kc + 1) * 128, :])
            sb_acc = persist.tile((128, TILES, D), fp32)
            sb_res = persist.tile((128, TILES, D), bf16)
            sb_eps = persist.tile((128, 1), fp32)
            nc.vector.memset(sb_eps[:], LN_EPS)
            sb_iotac = persist.tile((128, 1), fp32)
            nc.sync.dma_start(sb_iotac[:], d_iotac[:, :])
            sb_ad = persist.tile((128, TILES, HOPS, 8), bf16)

            # ---------- phase 0a: h0T full (replicated) ----------
            SUP = 512
            with tc.tile_pool(name="p0", bufs=3) as p0, \
                 tc.tile_pool(name="p0ps", bufs=4, space="PSUM") as p0ps:
                for st in range(NP // SUP):
                    xt = p0.tile((128, 2, SUP), bf16, tag="xt")
                    for kc in range(2):
                        nc.sync.dma_start(xt[:, kc, :],
                                          d_xT[kc * 128:(kc + 1) * 128, st * SUP:(st + 1) * SUP])
                    for mc in range(2):
                        ps = p0ps.tile((128, SUP), fp32, tag="ps")
                        for kc in range(2):
                            nc.tensor.matmul(ps[:], lhsT=sb_lin1[:, kc, mc * 128:(mc + 1) * 128],
                                             rhs=xt[:, kc, :], start=(kc == 0), stop=(kc == 1))
                        r = p0.tile((128, SUP), fp32, tag="r")
                        nc.scalar.activation(r[:], ps[:], AF.Relu, scale=1.0 - SLOPE_ACT)
                        h0 = p0.tile((128, SUP), bf16, tag="h0")
                        nc.vector.tensor_scalar(h0[:], ps[:], SLOPE_ACT, None, OP.mult)
                        nc.vector.tensor_add(h0[:], h0[:], r[:])
                        nc.sync.dma_start(d_hT0[mc, :, st * SUP:(st + 1) * SUP], h0[:])

            # ---------- phase 0b: residual h0 for own shard ----------
            with tc.tile_pool(name="p0b", bufs=3) as p0b, \
                 tc.tile_pool(name="p0bps", bufs=2, space="PSUM") as p0bps:
                for t in range(TILES):
                    xs = p0b.tile((128, 2, 128), bf16, tag="xs")
                    for kc in range(2):
                        nc.sync.dma_start(xs[:, kc, :], d_xTs[kc, :, t * 128:(t + 1) * 128])
                    ps = p0bps.tile((128, D), fp32, tag="ps")
                    for kc in range(2):
                        nc.tensor.matmul(ps[:], lhsT=xs[:, kc, :], rhs=sb_lin1[:, kc, :],
                                         start=(kc == 0), stop=(kc == 1))
                    r = p0b.tile((128, D), fp32, tag="r")
                    nc.scalar.activation(r[:], ps[:], AF.Relu, scale=1.0 - SLOPE_ACT)
                    t1 = p0b.tile((128, D), fp32, tag="t1")
                    nc.vector.tensor_scalar(t1[:], ps[:], SLOPE_ACT, None, OP.mult)
                    nc.vector.tensor_add(sb_res[:, t, :], t1[:], r[:])

            # ================= layers =================
            for l in range(LAYERS if stage >= 5 else 1):
                # ---- alpha_dst per own-shard tile ----
                with tc.tile_pool(name=f"ad{l}", bufs=3) as pad, \
                     tc.tile_pool(name=f"adps{l}", bufs=4, space="PSUM") as padps:
                    for t in range(TILES):
                        hts = pad.tile((128, 2, 128), bf16, tag="hts")
                        if l == 0:
                            xs2 = pad.tile((128, 2, 128), bf16, tag="xs2")
                            for kc in range(2):
                                nc.sync.dma_start(xs2[:, kc, :], d_xTs[kc, :, t * 128:(t + 1) * 128])
                            for mc in range(2):
                                pst = padps.tile((128, 128), fp32, tag="pst")
                                for kc in range(2):
                                    nc.tensor.matmul(pst[:], lhsT=sb_lin1[:, kc, mc * 128:(mc + 1) * 128],
                                                     rhs=xs2[:, kc, :], start=(kc == 0), stop=(kc == 1))
                                rr = pad.tile((128, 128), fp32, tag="rr")
                                nc.scalar.activation(rr[:], pst[:], AF.Relu, scale=1.0 - SLOPE_ACT)
                                tt = pad.tile((128, 128), fp32, tag="tt")
                                nc.vector.tensor_scalar(tt[:], pst[:], SLOPE_ACT, None, OP.mult)
                                nc.vector.tensor_add(hts[:, mc, :], tt[:], rr[:])
                        else:
                            for kc in range(2):
                                nc.sync.dma_start(hts[:, kc, :], d_agin[kc, :, t * 128:(t + 1) * 128])
                        psa = padps.tile((128, HOPS * 8), fp32, tag="psa")
                        for k in range(HOPS):
                            for kc in range(2):
                                nc.tensor.matmul(psa[:, k * 8:(k + 1) * 8],
                                                 lhsT=hts[:, kc, :],
                                                 rhs=sb_wcat[:, l, k, kc, D + HEADS:D + 2 * HEADS],
                                                 start=(kc == 0), stop=(kc == 1),
                                                 skip_group_check=True)
                        nc.vector.tensor_copy(
                            sb_ad[:, t, :, :].rearrange("p k h -> p (k h)"), psa[:])

                # ---- tables for all 3 hops ----
                if stage < 1:
                    break
                with tc.tile_pool(name=f"tb{l}", bufs=2) as ptb, \
                     tc.tile_pool(name=f"tbps{l}", bufs=2, space="PSUM") as ptbps:
                    for g in range(NT // GT):
                        n0 = g * GT * 128
                        hTc = ptb.tile((128, 2, GT, 128), bf16, tag="hTc")
                        for kc in range(2):
                            if l == 0:
                                nc.sync.dma_start(
                                    hTc[:, kc, :, :].rearrange("p g c -> p (g c)"),
                                    d_hT0[kc, :, n0:n0 + GT * 128])
                            else:
                                s = (g * GT) // TILES
                                j0 = (g * GT) % TILES
                                nc.sync.dma_start(
                                    hTc[:, kc, :, :].rearrange("p g c -> p (g c)"),
                                    d_agout[s, kc, :, j0 * 128:j0 * 128 + GT * 128])
                        stg = [ptb.tile((128, GT, TCOLS), bf16, tag=f"st{k}", name=f"stg{l}_{g}_{k}") for k in range(HOPS)]
                        for ci in range(GT):
                            pks = [ptbps.tile((128, TCOLS), fp32, tag=f"tp{k}", name=f"pks{l}_{g}_{ci}_{k}") for k in range(HOPS)]
                            for kc in range(2):
                                for k in range(HOPS):
                                    nc.tensor.matmul(pks[k][:], lhsT=hTc[:, kc, ci, :],
                                                     rhs=sb_wcat[:, l, k, kc, :],
                                                     start=(kc == 0), stop=(kc == 1),
                                                     skip_group_check=True)
                            for k in range(HOPS):
                                if k == 1:
                                    nc.scalar.copy(stg[k][:, ci, :], pks[k][:])
                                else:
                                    nc.vector.tensor_copy(stg[k][:, ci, :], pks[k][:])
                        for k in range(HOPS):
                            nc.sync.dma_start(
                                d_tab[k][n0:n0 + GT * 128, 0:TCOLS]
                                .rearrange("(g p) c -> p g c", p=128),
                                stg[k][:])

                # ---- edge phase: 3 hops ----
                nhops = HOPS if stage >= 3 else (1 if stage >= 2 else 0)
                for k in range(nhops):
                    dl_k = d_gidx[k]
                    with tc.tile_pool(name=f"eg{l}{k}", bufs=2) as peg, \
                         tc.tile_pool(name=f"egsc{l}{k}", bufs=2, space="PSUM") as psc, \
                         tc.tile_pool(name=f"egtr{l}{k}", bufs=2, space="PSUM") as ptr, \
                         tc.tile_pool(name=f"egdc{l}{k}", bufs=2, space="PSUM") as pdc:
                        for grp in groups:
                            t0 = grp[0]
                            gnL = int(sum(nch[k, t, 0] for t in grp))
                            gnH = int(sum(nch[k, t, 1] for t in grp))
                            gnc = int(sum(nch[k, t, 0] + nch[k, t, 1] for t in grp))
                            oL = lcum[k][t0]
                            oH = hcum[k][t0]
                            oc = col0[k][t0]

                            GSL = 28  # chunks per dma_gather
                            ixL = peg.tile((128, gnL * 8), i16, tag="ixL")
                            nc.sync.dma_start(ixL[:], dl_k[0][:, oL * 8:(oL + gnL) * 8])
                            gbL = peg.tile((128, gnL, ROWB), bf16, tag="gbL")
                            for off in range(0, gnL, GSL):
                                cnt = min(GSL, gnL - off)
                                nc.gpsimd.dma_gather(
                                    out_ap=gbL[:, off:off + cnt, :],
                                    in_ap=d_tab[k][0:LH_SPLIT, :],
                                    idxs_ap=ixL[:, off * 8:(off + cnt) * 8],
                                    num_idxs=cnt * 128,
                                    num_idxs_reg=cnt * 128, elem_size=ROWB,
                                    single_packet=False)
                            ixH = peg.tile((128, gnH * 8), i16, tag="ixH")
                            nc.sync.dma_start(ixH[:], dl_k[1][:, oH * 8:(oH + gnH) * 8])
                            gbH = peg.tile((128, gnH, ROWB), bf16, tag="gbH")
                            for off in range(0, gnH, GSL):
                                cnt = min(GSL, gnH - off)
                                nc.gpsimd.dma_gather(
                                    out_ap=gbH[:, off:off + cnt, :],
                                    in_ap=d_tab[k][LH_SPLIT:NP, :],
                                    idxs_ap=ixH[:, off * 8:(off + cnt) * 8],
                                    num_idxs=cnt * 128,
                                    num_idxs_reg=cnt * 128, elem_size=ROWB,
                                    single_packet=False)
                            dls = peg.tile((128, gnc), fp32, tag="dls")
                            nc.sync.dma_start(dls[:], d_dstloc[k][:, oc:oc + gnc])
                            mbs = peg.tile((128, gnc), fp32, tag="mbs")
                            nc.sync.dma_start(mbs[:], d_maskb[k][:, oc:oc + gnc])
                            drow = peg.tile((128, gnc * 128), bf16, tag="drow")
                            nc.sync.dma_start(drow[0:1, :], d_dstrow[k][0:1, oc * 128:(oc + gnc) * 128])
                            nc.gpsimd.partition_broadcast(drow[:], drow[0:1, :])

                            for t in grp:
                                if edgelvl < 2:
                                    continue
                                nL = int(nch[k, t, 0])
                                nH = int(nch[k, t, 1])
                                nT = nL + nH
                                lj = lcum[k][t] - oL
                                hj = hcum[k][t] - oH
                                cj = col0[k][t] - oc

                                V = peg.tile((128, nT, 264), bf16, tag="V")
                                e0 = peg.tile((128, nT, 8), fp32, tag="e0")
                                e1 = peg.tile((128, nT, 8), fp32, tag="e1")
                                PT = peg.tile((128, nT * 128), bf16, tag="PT")
                                nc.vector.tensor_scalar(
                                    PT[:], drow[:, cj * 128:(cj + nT) * 128],
                                    sb_iotac[:, 0:1], None, OP.is_equal)
                                pad_ps = psc.tile((128, nT, 8), fp32, tag="pad_ps")
                                for j in range(nT):
                                    nc.tensor.matmul(pad_ps[:, j, :],
                                                     lhsT=PT[:, j * 128:(j + 1) * 128],
                                                     rhs=sb_ad[:, t, k, :],
                                                     start=True, stop=True,
                                                     skip_group_check=True)
                                nc.vector.tensor_tensor(
                                    e0[:, 0:nL, :], gbL[:, lj:lj + nL, 256:264],
                                    pad_ps[:, 0:nL, :], op=OP.add)
                                nc.vector.tensor_tensor(
                                    e0[:, nL:nT, :], gbH[:, hj:hj + nH, 256:264],
                                    pad_ps[:, nL:nT, :], op=OP.add)
                                nc.vector.tensor_scalar(e1[:], e0[:], SLOPE_ATT, None, OP.mult)
                                nc.vector.tensor_tensor(e1[:], e0[:], e1[:], op=OP.max)
                                nc.vector.tensor_tensor(
                                    e1[:], e1[:],
                                    mbs[:, cj:cj + nT].to_broadcast((128, nT, 8)),
                                    op=OP.add)
                                nc.scalar.activation(V[:, :, 256:264], e1[:], AF.Exp)
                                if edgelvl < 3:
                                    continue
                                for j in range(nT):
                                    src = (gbL[:, lj + j, 0:256] if j < nL
                                           else gbH[:, hj + (j - nL), 0:256])
                                    nc.vector.tensor_tensor(
                                        V[:, j, 0:256].rearrange("p (h c) -> p h c", h=HEADS),
                                        src.rearrange("p (h c) -> p h c", h=HEADS),
                                        V[:, j, 256:264].to_broadcast((128, HEADS, DH)),
                                        op=OP.mult)
                                if edgelvl < 4:
                                    continue
                                P = peg.tile((128, nT, 128), bf16, tag="P")
                                nc.vector.tensor_tensor(
                                    P[:], dls[:, cj:cj + nT].to_broadcast((128, nT, 128)),
                                    bcast_mid(sb_iota[:], nT), op=OP.is_equal)
                                ps = psc.tile((128, 264), fp32, tag="ps")
                                for j in range(nT):
                                    nc.tensor.matmul(ps[:], lhsT=P[:, j, :], rhs=V[:, j, :],
                                                     start=(j == 0), stop=(j == nT - 1))
                                den = peg.tile((128, 8), fp32, tag="den")
                                nc.vector.tensor_scalar(den[:], ps[:, 256:264], 1e-16, None, OP.add)
                                rec = peg.tile((128, 8), fp32, tag="rec")
                                nc.vector.reciprocal(rec[:], den[:])
                                Gt = peg.tile((128, D), bf16, tag="Gt")
                                nc.vector.tensor_tensor(
                                    Gt[:].rearrange("p (h c) -> p h c", h=HEADS),
                                    ps[:, 0:256].rearrange("p (h c) -> p h c", h=HEADS),
                                    rec[:].to_broadcast((128, HEADS, DH)), op=OP.mult)
                                if edgelvl < 5:
                                    continue
                                GtT = peg.tile((128, 2, 128), bf16, tag="GtT")
                                for kc in range(2):
                                    pt = ptr.tile((128, 128), bf16, tag="pt")
                                    nc.tensor.transpose(pt[:], Gt[:, kc * 128:(kc + 1) * 128],
                                                        sb_ident[:])
                                    nc.vector.tensor_copy(GtT[:, kc, :], pt[:])
                                pd = pdc.tile((128, D), fp32, tag="pd")
                                for kc in range(2):
                                    nc.tensor.matmul(pd[:], lhsT=GtT[:, kc, :],
                                                     rhs=sb_dec[:, l, k, kc, :],
                                                     start=(kc == 0), stop=(kc == 1))
                                u1 = peg.tile((128, D), fp32, tag="u1")
                                nc.vector.tensor_scalar(u1[:], pd[:], DECAY[k], None, OP.mult)
                                u2 = peg.tile((128, D), fp32, tag="u2")
                                nc.scalar.activation(u2[:], pd[:], AF.Copy,
                                                     scale=DECAY[k] * SLOPE_ACT)
                                if k == 0:
                                    nc.vector.tensor_tensor(sb_acc[:, t, :], u1[:], u2[:], op=OP.max)
                                else:
                                    u3 = peg.tile((128, D), fp32, tag="u3")
                                    nc.vector.tensor_tensor(u3[:], u1[:], u2[:], op=OP.max)
                                    nc.vector.tensor_add(sb_acc[:, t, :], sb_acc[:, t, :], u3[:])

                # ---- layer norm + residual (+ agin / output) ----
                if stage < 3 or edgelvl < 5:
                    break
                with tc.tile_pool(name=f"ln{l}", bufs=3) as pln, \
                     tc.tile_pool(name=f"lnps{l}", bufs=2, space="PSUM") as plnps:
                    for t in range(TILES):
                        acc = sb_acc[:, t, :]
                        mu = pln.tile((128, 1), fp32, tag="mu")
                        nc.vector.reduce_sum(mu[:], acc, axis=mybir.AxisListType.X)
                        nc.vector.tensor_scalar(mu[:], mu[:], 1.0 / D, None, OP.mult)
                        xc = pln.tile((128, D), fp32, tag="xc")
                        nc.vector.tensor_scalar(xc[:], acc, mu[:, 0:1], None, OP.subtract)
                        sq = pln.tile((128, D), fp32, tag="sq")
                        nc.vector.tensor_tensor(sq[:], xc[:], xc[:], op=OP.mult)
                        var = pln.tile((128, 1), fp32, tag="var")
                        nc.vector.reduce_sum(var[:], sq[:], axis=mybir.AxisListType.X)
                        sd = pln.tile((128, 1), fp32, tag="sd")
                        nc.scalar.activation(sd[:], var[:], AF.Sqrt, bias=sb_eps[:], scale=1.0 / D)
                        nc.vector.reciprocal(sd[:], sd[:])
                        hn = pln.tile((128, D), fp32, tag="hn")
                        nc.vector.tensor_tensor(hn[:], xc[:], sd[:].to_broadcast((128, D)),
                                                op=OP.mult)
                        nc.vector.tensor_add(hn[:], hn[:], sb_res[:, t, :])
                        if l == 0:
                            nc.vector.tensor_copy(sb_res[:, t, :], hn[:])
                            for kc in range(2):
                                pt = plnps.tile((128, 128), bf16, tag="pt")
                                nc.tensor.transpose(pt[:], sb_res[:, t, kc * 128:(kc + 1) * 128],
                                                    sb_ident[:])
                                hb = pln.tile((128, 128), bf16, tag="hb")
                                nc.vector.tensor_copy(hb[:], pt[:])
                                nc.sync.dma_start(d_agin[kc, :, t * 128:(t + 1) * 128], hb[:])
                        else:
                            nc.sync.dma_start(d_out[t * 128:(t + 1) * 128, :], hn[:])

                if l == 0 and stage >= 4:
                    nc.gpsimd.collective_compute(
                        "AllGather", mybir.AluOpType.bypass,
                        replica_groups=[list(range(NCORES))],
                        ins=[d_agin[:, :, :]],
                        outs=[d_agout[:, :, :, :]],
                    )

            if stage < 5:
                with tc.tile_pool(name="dbg", bufs=2) as pdbg:
                    for t in range(TILES):
                        z = pdbg.tile((128, D), fp32, tag="z")
                        nc.vector.tensor_copy(z[:], sb_res[:, t, :])
                        nc.sync.dma_start(d_out[t * 128:(t + 1) * 128, :], z[:])

    nc.compile()
    return nc


def kernel(**inputs):
    import os, sys, time
    from concourse.bass_utils import run_bass_kernel_spmd

    t0 = time.perf_counter()
    prep = _host_prep(inputs)
    t1 = time.perf_counter()
    print(f"[kernel] host_prep: {t1 - t0:.2f}s", file=sys.stderr, flush=True)
    nc = _build(prep, stage=int(os.environ.get("GAT_STAGE", "5")), edgelvl=int(os.environ.get("GAT_EDGELVL", "5")))
    t2 = time.perf_counter()
    print(f"[kernel] build+compile: {t2 - t1:.2f}s", file=sys.stderr, flush=True)

    in_maps = []
    for c in range(NCORES):
        m = {
            "xT": prep["xT"], "xTs": np.ascontiguousarray(prep["xTs"][c]),
            "lin1": prep["lin1"], "wcat": prep["wcat"], "dec": prep["dec"],
            "iota": prep["iota"], "iotac": prep["iotac"],
        }
        for k in range(HOPS):
            cd = prep["core_data"][c][k]
            m[f"gidxL{k}"] = cd["Lidx"]
            m[f"gidxH{k}"] = cd["Hidx"]
            m[f"dstloc{k}"] = cd["dstloc"]
            m[f"dstrow{k}"] = cd["dstrow"]
            m[f"maskb{k}"] = cd["maskb"]
        in_maps.append(m)

    t3 = time.perf_counter()
    print(f"[kernel] in_maps: {t3 - t2:.2f}s", file=sys.stderr, flush=True)
    res = run_bass_kernel_spmd(nc, in_maps, core_ids=list(range(NCORES)))
    t4 = time.perf_counter()
    print(f"[kernel] run_spmd: {t4 - t3:.2f}s", file=sys.stderr, flush=True)
    kernel.last_exec_ns = res.exec_time_ns

    out = np.zeros((N, D), np.float32)
    for c in range(NCORES):
        out[c * SHARD:(c + 1) * SHARD] = res.results[c]["out"][:SHARD]
    print(f"[kernel] post: {time.perf_counter() - t4:.2f}s", file=sys.stderr, flush=True)
    return out



# revision 7
# speedup vs baseline: 2.2506x; 2.2506x over previous
"""GAT-KH (2-layer, 3-hop, 8-head GAT, N=50k, E=300k/hop) on 8 TRN2 cores.

Distribution: dst-sharded edges, sharded gather-table build + AllGather.
- Nodes renumbered into 8 padded shards of 6400 (NP=51200). Core c owns dst
  shard c (tiles of 128 dsts, 50 tiles).
- Per layer: core c computes h^T for its own shard (SBUF-resident), builds
  its shard's slice of the 3 per-hop gather tables T_k[n] = [hp(256) |
  alpha_src(8)] bf16 (768B row stride), then AllGathers each table so every
  core holds all NP rows.
- Edge phase: per dst tile, edges chunked by 128 (split src<32768 for int16
  dma_gather), gathered rows -> softmax numerators w=exp(leaky(as+ad)+mb)
  -> one-hot P matmuls scatter [w*hp | w] into PSUM -> normalize -> dec_w
  -> leaky -> decay-accumulate.
- Layer end: LayerNorm+residual; h1^T transposed back into SBUF for layer-2
  tables. Output: per-core h shard (bf16), host-concatenated.
- Runner: inputs are device_put asynchronously (sharded over the 8 cores)
  while the bass build + NEFF compile run, then executed via PJRT.
"""

import numpy as np
import ml_dtypes
from contextlib import ExitStack

N = 50000
E = 300000
HOPS = 3
LAYERS = 2
HEADS = 8
D = 256
DH = 32
NCORES = 8
SHARD = 6250
SHARD_P = 6400
TILES = SHARD_P // 128            # 50
NP = NCORES * SHARD_P             # 51200
ROWB = 384                        # table row length in bf16 elems (768B)
TCOLS = 272                       # wcat cols: [W(256) | a_src(8) | a_dst(8)]
TSTORE = 264                      # table cols actually stored: hp + alpha_src
LH_SPLIT = 32768
DECAY = [float(np.exp(-0.5 * k)) for k in range(HOPS)]
SLOPE_ACT = 0.01
SLOPE_ATT = 0.2
LN_EPS = 1e-5
NEG_BIAS = -30000.0
TPG = 3                           # dst tiles per gather group
GT = 10                           # table chunks per staging group (50%GT==0)
BF16 = ml_dtypes.bfloat16


def _pack_idx16(idx):
    """int16 idx list -> [128, ceil(n/16)] wrapped in 16 partitions, x8."""
    n = len(idx)
    n16 = max(1, (n + 15) // 16)
    pad = np.zeros(n16 * 16, np.int16)
    pad[:n] = idx
    a = pad.reshape(n16, 16).T.copy()
    return np.tile(a, (8, 1))


def _host_prep(inputs):
    x = np.asarray(inputs["x"], np.float32)
    ei = np.asarray(inputs["edge_index_k_hops"])
    lin1_w = np.asarray(inputs["lin1_w"], np.float32)
    gat_w = np.asarray(inputs["gat_w"], np.float32)
    a_src = np.asarray(inputs["gat_att_src"], np.float32)
    a_dst = np.asarray(inputs["gat_att_dst"], np.float32)
    dec_w = np.asarray(inputs["dec_w"], np.float32)

    wcat = np.zeros((LAYERS, HOPS, D, TCOLS), np.float32)
    Wh = gat_w.reshape(LAYERS, HOPS, D, HEADS, DH)
    wcat[:, :, :, :D] = gat_w
    wcat[:, :, :, D:D + HEADS] = np.einsum("lkdhc,lkhc->lkdh", Wh, a_src)
    wcat[:, :, :, D + HEADS:] = np.einsum("lkdhc,lkhc->lkdh", Wh, a_dst)

    xT = np.zeros((D, NP), np.float32)
    xTfull = xT.reshape(D, NCORES, SHARD_P)
    xTfull[:, :, :SHARD] = x.reshape(NCORES, SHARD, D).transpose(2, 0, 1)
    xT_bf = xT.astype(BF16)
    xTs = [xT_bf[:, c * SHARD_P:(c + 1) * SHARD_P].reshape(2, 128, SHARD_P)
           for c in range(NCORES)]

    # ---- edge routing (vectorized) ----
    cnts = np.zeros((HOPS, NCORES, TILES, 2), np.int64)
    hop_sorted = []
    for k in range(HOPS):
        src = ei[k, 0].astype(np.int64)
        dst = ei[k, 1].astype(np.int64)
        ps = (src // SHARD) * SHARD_P + (src % SHARD)
        core = dst // SHARD
        dl = dst % SHARD
        tl = dl // 128
        dloc = dl % 128
        side = (ps >= LH_SPLIT).astype(np.int64)
        keyfull = (core * TILES + tl) * 2 + side
        order = np.argsort(keyfull, kind="stable")
        cnt = np.bincount(keyfull, minlength=NCORES * TILES * 2)
        cnts[k] = cnt.reshape(NCORES, TILES, 2)
        hop_sorted.append((keyfull[order], ps[order], dloc[order], side[order], cnt))
    nch = np.maximum(1, np.ceil(cnts.max(axis=1) / 128.0)).astype(np.int64)

    core_data = [[] for _ in range(NCORES)]
    for k in range(HOPS):
        kf_s, ps_s, dloc_s, side_s, cnt = hop_sorted[k]
        cap_flat = (nch[k].reshape(-1) * 128).astype(np.int64)   # per (t,side)
        bucket_off = np.zeros(TILES * 2, np.int64)
        bucket_off[1:] = np.cumsum(cap_flat)[:-1]
        total_cap = int(cap_flat.sum())                          # NCH[k]*128
        starts = np.zeros(NCORES * TILES * 2, np.int64)
        starts[1:] = np.cumsum(cnt)[:-1]
        rank = np.arange(E, dtype=np.int64) - starts[kf_s]
        dest = (kf_s // (TILES * 2)) * total_cap + bucket_off[kf_s % (TILES * 2)] + rank

        idxA = np.zeros(NCORES * total_cap, np.int64)
        dvA = np.zeros(NCORES * total_cap, np.int64)
        mbA = np.full(NCORES * total_cap, NEG_BIAS, np.float32)
        idxA[dest] = ps_s - side_s * LH_SPLIT
        dvA[dest] = dloc_s
        mbA[dest] = 0.0

        Lsel = np.concatenate([np.arange(bucket_off[2 * t], bucket_off[2 * t] + cap_flat[2 * t])
                               for t in range(TILES)])
        Hsel = np.concatenate([np.arange(bucket_off[2 * t + 1], bucket_off[2 * t + 1] + cap_flat[2 * t + 1])
                               for t in range(TILES)])
        for c in range(NCORES):
            blk = slice(c * total_cap, (c + 1) * total_cap)
            idxc = idxA[blk]
            dvc = dvA[blk]
            nchk = total_cap // 128
            core_data[c].append({
                "Lidx": _pack_idx16(idxc[Lsel].astype(np.int16)),
                "Hidx": _pack_idx16(idxc[Hsel].astype(np.int16)),
                "dstloc": dvc.reshape(nchk, 128).T.astype(np.float32),
                "dstrow": dvc.astype(np.float32).astype(BF16).reshape(1, -1),
                "maskb": mbA[blk].reshape(nchk, 128).T.copy(),
            })

    iota = np.tile(np.arange(128, dtype=np.float32)[None, :], (128, 1))
    iotac = np.ascontiguousarray(np.arange(128, dtype=np.float32)[:, None])
    return {
        "iotac": iotac,
        "wcat": wcat.astype(BF16), "xTs": xTs,
        "lin1": lin1_w.astype(BF16), "dec": dec_w.astype(BF16),
        "core_data": core_data, "nch": nch, "iota": iota,
    }


def _build(prep, stage=5, edgelvl=5):
    from concourse import bass, mybir, tile, library_config
    from concourse.bass import AP
    from concourse.masks import make_identity
    import concourse.bacc as bacc

    nch = prep["nch"]
    cd0 = prep["core_data"][0]
    NCH = [cd0[k]["dstloc"].shape[1] for k in range(HOPS)]
    NIDXL = [cd0[k]["Lidx"].shape[1] * 16 for k in range(HOPS)]
    NIDXH = [cd0[k]["Hidx"].shape[1] * 16 for k in range(HOPS)]

    fp32 = mybir.dt.float32
    bf16 = mybir.dt.bfloat16
    i16 = mybir.dt.int16
    AF = mybir.ActivationFunctionType
    OP = mybir.AluOpType

    nc = bacc.Bacc("TRN2", target_bir_lowering=False, debug=False,
                   num_devices=NCORES)

    d_xTs = nc.dram_tensor("xTs", [2, 128, SHARD_P], bf16, kind="ExternalInput")
    d_lin1 = nc.dram_tensor("lin1", [D, D], bf16, kind="ExternalInput")
    d_wcat = nc.dram_tensor("wcat", [LAYERS, HOPS, D, TCOLS], bf16, kind="ExternalInput")
    d_dec = nc.dram_tensor("dec", [LAYERS, HOPS, D, D], bf16, kind="ExternalInput")
    d_iota = nc.dram_tensor("iota", [128, 128], fp32, kind="ExternalInput")
    d_gidx = [(nc.dram_tensor(f"gidxL{k}", [128, NIDXL[k] // 16], i16, kind="ExternalInput"),
               nc.dram_tensor(f"gidxH{k}", [128, NIDXH[k] // 16], i16, kind="ExternalInput"))
              for k in range(HOPS)]
    d_dstloc = [nc.dram_tensor(f"dstloc{k}", [128, NCH[k]], fp32, kind="ExternalInput") for k in range(HOPS)]
    d_dstrow = [nc.dram_tensor(f"dstrow{k}", [1, NCH[k] * 128], bf16, kind="ExternalInput") for k in range(HOPS)]
    d_iotac = nc.dram_tensor("iotac", [128, 1], fp32, kind="ExternalInput")
    d_maskb = [nc.dram_tensor(f"maskb{k}", [128, NCH[k]], fp32, kind="ExternalInput") for k in range(HOPS)]
    d_out = nc.dram_tensor("out", [SHARD_P, D], bf16, kind="ExternalOutput")

    d_tabin = [nc.dram_tensor(f"tabin{k}", [SHARD_P, ROWB], bf16, kind="Internal")
               for k in range(HOPS)]
    d_tab = [nc.dram_tensor(f"tab{k}", [NP, ROWB], bf16, kind="Internal",
                            addr_space="Shared") for k in range(HOPS)]

    # per-hop static chunk bookkeeping
    col0 = []   # [k][t] first chunk col of tile t
    lcum = []   # [k][t] first L-chunk index (global, within hop) of tile t
    hcum = []
    for k in range(HOPS):
        c0, lc, hc = [], [], []
        a = b = g = 0
        for t in range(TILES):
            c0.append(g)
            lc.append(a)
            hc.append(b)
            a += int(nch[k, t, 0])
            b += int(nch[k, t, 1])
            g += int(nch[k, t, 0] + nch[k, t, 1])
        col0.append(c0)
        lcum.append(lc)
        hcum.append(hc)

    groups = [list(range(g0, min(g0 + TPG, TILES))) for g0 in range(0, TILES, TPG)]

    def bcast_mid(ap, n):
        """[128, M] AP -> [128, n, M] with 0-step middle dim."""
        return AP(ap.tensor, ap.offset, [list(ap.ap[0]), [0, n], list(ap.ap[1])])

    with tile.TileContext(nc) as tc:
        with ExitStack() as ctx:
            persist = ctx.enter_context(tc.tile_pool(name="persist", bufs=1))
            nc.gpsimd.load_library(library_config.mlp)

            sb_iota = persist.tile((128, 128), fp32)
            nc.sync.dma_start(sb_iota[:], d_iota[:, :])
            sb_ident = persist.tile((128, 128), bf16)
            make_identity(nc, sb_ident[:])
            sb_lin1 = persist.tile((128, 2, D), bf16)
            for kc in range(2):
                nc.sync.dma_start(sb_lin1[:, kc, :], d_lin1[kc * 128:(kc + 1) * 128, :])
            sb_wcat = persist.tile((128, LAYERS, HOPS, 2, TCOLS), bf16)
            sb_dec = persist.tile((128, LAYERS, HOPS, 2, D), bf16)
            for l in range(LAYERS):
                for k in range(HOPS):
                    for kc in range(2):
                        nc.sync.dma_start(sb_wcat[:, l, k, kc, :],
                                          d_wcat[l, k, kc * 128:(kc + 1) * 128, :])
                        nc.sync.dma_start(sb_dec[:, l, k, kc, :],
                                          d_dec[l, k, kc * 128:(kc + 1) * 128, :])
            sb_acc = persist.tile((128, TILES, D), fp32)
            sb_res = persist.tile((128, TILES, D), bf16)
            sb_eps = persist.tile((128, 1), fp32)
            nc.vector.memset(sb_eps[:], LN_EPS)
            sb_iotac = persist.tile((128, 1), fp32)
            nc.sync.dma_start(sb_iotac[:], d_iotac[:, :])
            sb_ad = persist.tile((128, TILES, HOPS, 8), bf16)
            sb_hT = persist.tile((128, 2, SHARD_P), bf16)

            # ---------- phase 0: h0^T for own shard (SBUF-resident) ----------
            with tc.tile_pool(name="p0", bufs=1) as p0x, \
                 tc.tile_pool(name="p0w", bufs=3) as p0, \
                 tc.tile_pool(name="p0ps", bufs=4, space="PSUM") as p0ps:
                sb_xT = p0x.tile((128, 2, SHARD_P), bf16)
                for kc in range(2):
                    nc.sync.dma_start(sb_xT[:, kc, :], d_xTs[kc, :, :])
                SUP = 512
                spans = [(s, min(SUP, SHARD_P - s)) for s in range(0, SHARD_P, SUP)]
                for (s0, w) in spans:
                    for mc in range(2):
                        ps = p0ps.tile((128, SUP), fp32, tag="ps")
                        for kc in range(2):
                            nc.tensor.matmul(ps[:, 0:w],
                                             lhsT=sb_lin1[:, kc, mc * 128:(mc + 1) * 128],
                                             rhs=sb_xT[:, kc, s0:s0 + w],
                                             start=(kc == 0), stop=(kc == 1))
                        r = p0.tile((128, SUP), fp32, tag="r")
                        nc.scalar.activation(r[:, 0:w], ps[:, 0:w], AF.Relu,
                                             scale=1.0 - SLOPE_ACT)
                        h0 = p0.tile((128, SUP), fp32, tag="h0")
                        nc.vector.tensor_scalar(h0[:, 0:w], ps[:, 0:w], SLOPE_ACT, None, OP.mult)
                        nc.vector.tensor_add(sb_hT[:, mc, s0:s0 + w], h0[:, 0:w], r[:, 0:w])

            # ---------- phase 0b: residual h0 rows from sb_hT ----------
            with tc.tile_pool(name="p0b", bufs=3) as p0b, \
                 tc.tile_pool(name="p0bps", bufs=4, space="PSUM") as p0bps:
                for t in range(TILES):
                    for kc in range(2):
                        pt = p0bps.tile((128, 128), bf16, tag="pt")
                        nc.tensor.transpose(pt[:], sb_hT[:, kc, t * 128:(t + 1) * 128],
                                            sb_ident[:])
                        nc.vector.tensor_copy(sb_res[:, t, kc * 128:(kc + 1) * 128], pt[:])

            # ================= layers =================
            for l in range(LAYERS if stage >= 5 else 1):
                # ---- alpha_dst per own-shard tile ----
                with tc.tile_pool(name=f"ad{l}", bufs=3) as pad, \
                     tc.tile_pool(name=f"adps{l}", bufs=4, space="PSUM") as padps:
                    for t in range(TILES):
                        psa = padps.tile((128, HOPS * 8), fp32, tag="psa")
                        for k in range(HOPS):
                            for kc in range(2):
                                nc.tensor.matmul(psa[:, k * 8:(k + 1) * 8],
                                                 lhsT=sb_hT[:, kc, t * 128:(t + 1) * 128],
                                                 rhs=sb_wcat[:, l, k, kc, D + HEADS:D + 2 * HEADS],
                                                 start=(kc == 0), stop=(kc == 1),
                                                 skip_group_check=True)
                        nc.vector.tensor_copy(
                            sb_ad[:, t, :, :].rearrange("p k h -> p (k h)"), psa[:])

                # ---- own-shard tables for all 3 hops, then AllGather ----
                if stage < 1:
                    break
                with tc.tile_pool(name=f"tb{l}", bufs=2) as ptb, \
                     tc.tile_pool(name=f"tbps{l}", bufs=2, space="PSUM") as ptbps:
                    for g in range(TILES // GT):
                        n0 = g * GT * 128
                        stg = [ptb.tile((128, GT, TSTORE), bf16, tag=f"st{k}",
                                        name=f"stg{l}_{g}_{k}") for k in range(HOPS)]
                        for ci in range(GT):
                            cs = n0 + ci * 128
                            pks = [ptbps.tile((128, TSTORE), fp32, tag=f"tp{k}",
                                              name=f"pks{l}_{g}_{ci}_{k}") for k in range(HOPS)]
                            for kc in range(2):
                                for k in range(HOPS):
                                    nc.tensor.matmul(pks[k][:],
                                                     lhsT=sb_hT[:, kc, cs:cs + 128],
                                                     rhs=sb_wcat[:, l, k, kc, 0:TSTORE],
                                                     start=(kc == 0), stop=(kc == 1),
                                                     skip_group_check=True)
                            for k in range(HOPS):
                                if k == 1:
                                    nc.scalar.copy(stg[k][:, ci, :], pks[k][:])
                                else:
                                    nc.vector.tensor_copy(stg[k][:, ci, :], pks[k][:])
                        for k in range(HOPS):
                            nc.sync.dma_start(
                                d_tabin[k][n0:n0 + GT * 128, 0:TSTORE]
                                .rearrange("(g p) c -> p g c", p=128),
                                stg[k][:])
                for k in range(HOPS):
                    nc.gpsimd.collective_compute(
                        "AllGather", mybir.AluOpType.bypass,
                        replica_groups=[list(range(NCORES))],
                        ins=[d_tabin[k][:, :]],
                        outs=[d_tab[k][:, :]],
                    )

                # ---- edge phase: 3 hops ----
                nhops = HOPS if stage >= 3 else (1 if stage >= 2 else 0)
                for k in range(nhops):
                    dl_k = d_gidx[k]
                    with tc.tile_pool(name=f"eg{l}{k}", bufs=2) as peg, \
                         tc.tile_pool(name=f"egsc{l}{k}", bufs=2, space="PSUM") as psc, \
                         tc.tile_pool(name=f"egtr{l}{k}", bufs=2, space="PSUM") as ptr, \
                         tc.tile_pool(name=f"egdc{l}{k}", bufs=2, space="PSUM") as pdc:
                        for grp in groups:
                            t0 = grp[0]
                            gnL = int(sum(nch[k, t, 0] for t in grp))
                            gnH = int(sum(nch[k, t, 1] for t in grp))
                            gnc = int(sum(nch[k, t, 0] + nch[k, t, 1] for t in grp))
                            oL = lcum[k][t0]
                            oH = hcum[k][t0]
                            oc = col0[k][t0]

                            GSL = 28  # chunks per dma_gather
                            ixL = peg.tile((128, gnL * 8), i16, tag="ixL")
                            nc.sync.dma_start(ixL[:], dl_k[0][:, oL * 8:(oL + gnL) * 8])
                            gbL = peg.tile((128, gnL, ROWB), bf16, tag="gbL")
                            for off in range(0, gnL, GSL):
                                cnt = min(GSL, gnL - off)
                                nc.gpsimd.dma_gather(
                                    out_ap=gbL[:, off:off + cnt, :],
                                    in_ap=d_tab[k][0:LH_SPLIT, :],
                                    idxs_ap=ixL[:, off * 8:(off + cnt) * 8],
                                    num_idxs=cnt * 128,
                                    num_idxs_reg=cnt * 128, elem_size=ROWB,
                                    single_packet=False)
                            ixH = peg.tile((128, gnH * 8), i16, tag="ixH")
                            nc.sync.dma_start(ixH[:], dl_k[1][:, oH * 8:(oH + gnH) * 8])
                            gbH = peg.tile((128, gnH, ROWB), bf16, tag="gbH")
                            for off in range(0, gnH, GSL):
                                cnt = min(GSL, gnH - off)
                                nc.gpsimd.dma_gather(
                                    out_ap=gbH[:, off:off + cnt, :],
                                    in_ap=d_tab[k][LH_SPLIT:NP, :],
                                    idxs_ap=ixH[:, off * 8:(off + cnt) * 8],
                                    num_idxs=cnt * 128,
                                    num_idxs_reg=cnt * 128, elem_size=ROWB,
                                    single_packet=False)
                            dls = peg.tile((128, gnc), fp32, tag="dls")
                            nc.sync.dma_start(dls[:], d_dstloc[k][:, oc:oc + gnc])
                            mbs = peg.tile((128, gnc), fp32, tag="mbs")
                            nc.sync.dma_start(mbs[:], d_maskb[k][:, oc:oc + gnc])
                            drow = peg.tile((128, gnc * 128), bf16, tag="drow")
                            nc.sync.dma_start(drow[0:1, :], d_dstrow[k][0:1, oc * 128:(oc + gnc) * 128])
                            nc.gpsimd.partition_broadcast(drow[:], drow[0:1, :])

                            for t in grp:
                                if edgelvl < 2:
                                    continue
                                nL = int(nch[k, t, 0])
                                nH = int(nch[k, t, 1])
                                nT = nL + nH
                                lj = lcum[k][t] - oL
                                hj = hcum[k][t] - oH
                                cj = col0[k][t] - oc

                                V = peg.tile((128, nT, 264), bf16, tag="V")
                                e0 = peg.tile((128, nT, 8), fp32, tag="e0")
                                e1 = peg.tile((128, nT, 8), fp32, tag="e1")
                                PT = peg.tile((128, nT * 128), bf16, tag="PT")
                                nc.vector.tensor_scalar(
                                    PT[:], drow[:, cj * 128:(cj + nT) * 128],
                                    sb_iotac[:, 0:1], None, OP.is_equal)
                                pad_ps = psc.tile((128, nT, 8), fp32, tag="pad_ps")
                                for j in range(nT):
                                    nc.tensor.matmul(pad_ps[:, j, :],
                                                     lhsT=PT[:, j * 128:(j + 1) * 128],
                                                     rhs=sb_ad[:, t, k, :],
                                                     start=True, stop=True,
                                                     skip_group_check=True)
                                nc.vector.tensor_tensor(
                                    e0[:, 0:nL, :], gbL[:, lj:lj + nL, 256:264],
                                    pad_ps[:, 0:nL, :], op=OP.add)
                                nc.vector.tensor_tensor(
                                    e0[:, nL:nT, :], gbH[:, hj:hj + nH, 256:264],
                                    pad_ps[:, nL:nT, :], op=OP.add)
                                nc.vector.tensor_scalar(e1[:], e0[:], SLOPE_ATT, None, OP.mult)
                                nc.vector.tensor_tensor(e1[:], e0[:], e1[:], op=OP.max)
                                nc.vector.tensor_tensor(
                                    e1[:], e1[:],
                                    mbs[:, cj:cj + nT].to_broadcast((128, nT, 8)),
                                    op=OP.add)
                                nc.scalar.activation(V[:, :, 256:264], e1[:], AF.Exp)
                                if edgelvl < 3:
                                    continue
                                for j in range(nT):
                                    src = (gbL[:, lj + j, 0:256] if j < nL
                                           else gbH[:, hj + (j - nL), 0:256])
                                    nc.vector.tensor_tensor(
                                        V[:, j, 0:256].rearrange("p (h c) -> p h c", h=HEADS),
                                        src.rearrange("p (h c) -> p h c", h=HEADS),
                                        V[:, j, 256:264].to_broadcast((128, HEADS, DH)),
                                        op=OP.mult)
                                if edgelvl < 4:
                                    continue
                                P = peg.tile((128, nT, 128), bf16, tag="P")
                                nc.vector.tensor_tensor(
                                    P[:], dls[:, cj:cj + nT].to_broadcast((128, nT, 128)),
                                    bcast_mid(sb_iota[:], nT), op=OP.is_equal)
                                ps = psc.tile((128, 264), fp32, tag="ps")
                                for j in range(nT):
                                    nc.tensor.matmul(ps[:], lhsT=P[:, j, :], rhs=V[:, j, :],
                                                     start=(j == 0), stop=(j == nT - 1))
                                den = peg.tile((128, 8), fp32, tag="den")
                                nc.vector.tensor_scalar(den[:], ps[:, 256:264], 1e-16, None, OP.add)
                                rec = peg.tile((128, 8), fp32, tag="rec")
                                nc.vector.reciprocal(rec[:], den[:])
                                Gt = peg.tile((128, D), bf16, tag="Gt")
                                nc.vector.tensor_tensor(
                                    Gt[:].rearrange("p (h c) -> p h c", h=HEADS),
                                    ps[:, 0:256].rearrange("p (h c) -> p h c", h=HEADS),
                                    rec[:].to_broadcast((128, HEADS, DH)), op=OP.mult)
                                if edgelvl < 5:
                                    continue
                                GtT = peg.tile((128, 2, 128), bf16, tag="GtT")
                                for kc in range(2):
                                    pt = ptr.tile((128, 128), bf16, tag="pt")
                                    nc.tensor.transpose(pt[:], Gt[:, kc * 128:(kc + 1) * 128],
                                                        sb_ident[:])
                                    nc.vector.tensor_copy(GtT[:, kc, :], pt[:])
                                pd = pdc.tile((128, D), fp32, tag="pd")
                                for kc in range(2):
                                    nc.tensor.matmul(pd[:], lhsT=GtT[:, kc, :],
                                                     rhs=sb_dec[:, l, k, kc, :],
                                                     start=(kc == 0), stop=(kc == 1))
                                u1 = peg.tile((128, D), fp32, tag="u1")
                                nc.vector.tensor_scalar(u1[:], pd[:], DECAY[k], None, OP.mult)
                                u2 = peg.tile((128, D), fp32, tag="u2")
                                nc.scalar.activation(u2[:], pd[:], AF.Copy,
                                                     scale=DECAY[k] * SLOPE_ACT)
                                if k == 0:
                                    nc.vector.tensor_tensor(sb_acc[:, t, :], u1[:], u2[:], op=OP.max)
                                else:
                                    u3 = peg.tile((128, D), fp32, tag="u3")
                                    nc.vector.tensor_tensor(u3[:], u1[:], u2[:], op=OP.max)
                                    nc.vector.tensor_add(sb_acc[:, t, :], sb_acc[:, t, :], u3[:])

                # ---- layer norm + residual (+ next-layer hT / output) ----
                if stage < 3 or edgelvl < 5:
                    break
                with tc.tile_pool(name=f"ln{l}", bufs=3) as pln, \
                     tc.tile_pool(name=f"lnps{l}", bufs=2, space="PSUM") as plnps:
                    for t in range(TILES):
                        acc = sb_acc[:, t, :]
                        mu = pln.tile((128, 1), fp32, tag="mu")
                        nc.vector.reduce_sum(mu[:], acc, axis=mybir.AxisListType.X)
                        nc.vector.tensor_scalar(mu[:], mu[:], 1.0 / D, None, OP.mult)
                        xc = pln.tile((128, D), fp32, tag="xc")
                        nc.vector.tensor_scalar(xc[:], acc, mu[:, 0:1], None, OP.subtract)
                        sq = pln.tile((128, D), fp32, tag="sq")
                        nc.vector.tensor_tensor(sq[:], xc[:], xc[:], op=OP.mult)
                        var = pln.tile((128, 1), fp32, tag="var")
                        nc.vector.reduce_sum(var[:], sq[:], axis=mybir.AxisListType.X)
                        sd = pln.tile((128, 1), fp32, tag="sd")
                        nc.scalar.activation(sd[:], var[:], AF.Sqrt, bias=sb_eps[:], scale=1.0 / D)
                        nc.vector.reciprocal(sd[:], sd[:])
                        hn = pln.tile((128, D), fp32, tag="hn")
                        nc.vector.tensor_tensor(hn[:], xc[:], sd[:].to_broadcast((128, D)),
                                                op=OP.mult)
                        nc.vector.tensor_add(hn[:], hn[:], sb_res[:, t, :])
                        if l == 0:
                            nc.vector.tensor_copy(sb_res[:, t, :], hn[:])
                            for kc in range(2):
                                pt = plnps.tile((128, 128), bf16, tag="pt")
                                nc.tensor.transpose(pt[:], sb_res[:, t, kc * 128:(kc + 1) * 128],
                                                    sb_ident[:])
                                nc.vector.tensor_copy(sb_hT[:, kc, t * 128:(t + 1) * 128], pt[:])
                        else:
                            hb = pln.tile((128, D), bf16, tag="hb")
                            nc.vector.tensor_copy(hb[:], hn[:])
                            nc.sync.dma_start(d_out[t * 128:(t + 1) * 128, :], hb[:])

            if stage < 5:
                with tc.tile_pool(name="dbg", bufs=2) as pdbg:
                    for t in range(TILES):
                        z = pdbg.tile((128, D), bf16, tag="z")
                        nc.vector.tensor_copy(z[:], sb_res[:, t, :])
                        nc.sync.dma_start(d_out[t * 128:(t + 1) * 128, :], z[:])

    nc.compile()
    return nc


IN_NAMES = (["xTs", "lin1", "wcat", "dec", "iota"]
            + [n for k in range(HOPS) for n in (f"gidxL{k}", f"gidxH{k}")]
            + [f"dstloc{k}" for k in range(HOPS)]
            + [f"dstrow{k}" for k in range(HOPS)]
            + ["iotac"] + [f"maskb{k}" for k in range(HOPS)]
            + ["out"])


def kernel(**inputs):
    import os, sys, time
    import jax
    from jax.sharding import Mesh, PartitionSpec, NamedSharding

    t0 = time.perf_counter()
    prep = _host_prep(inputs)
    t1 = time.perf_counter()
    print(f"[kernel] host_prep: {t1 - t0:.2f}s", file=sys.stderr, flush=True)

    # ---- start async sharded upload of all inputs ----
    n_cores = NCORES
    devices = jax.devices()[:n_cores]
    mesh = Mesh(np.asarray(devices), ("core",))
    sharding = NamedSharding(mesh, PartitionSpec("core"))

    def core_map(c):
        m = {"xTs": np.ascontiguousarray(prep["xTs"][c]),
             "lin1": prep["lin1"], "wcat": prep["wcat"], "dec": prep["dec"],
             "iota": prep["iota"], "iotac": prep["iotac"]}
        for k in range(HOPS):
            cd = prep["core_data"][c][k]
            m[f"gidxL{k}"] = cd["Lidx"]
            m[f"gidxH{k}"] = cd["Hidx"]
            m[f"dstloc{k}"] = cd["dstloc"]
            m[f"dstrow{k}"] = cd["dstrow"]
            m[f"maskb{k}"] = cd["maskb"]
        m["out"] = np.zeros((SHARD_P, D), BF16)
        return m

    maps = [core_map(c) for c in range(n_cores)]
    param_names = [n for n in IN_NAMES]
    dev_args = []
    for name in param_names:
        cat = np.concatenate([maps[c][name] for c in range(n_cores)], axis=0)
        dev_args.append(jax.device_put(cat, sharding))
    t2 = time.perf_counter()
    print(f"[kernel] upload started: {t2 - t1:.2f}s", file=sys.stderr, flush=True)

    nc = _build(prep, stage=int(os.environ.get("GAT_STAGE", "5")),
                edgelvl=int(os.environ.get("GAT_EDGELVL", "5")))
    t3 = time.perf_counter()
    print(f"[kernel] build+compile: {t3 - t2:.2f}s", file=sys.stderr, flush=True)

    # ---- PJRT execution (mirrors run_bass_via_pjrt, with device inputs) ----
    from jax.experimental.shard_map import shard_map
    from concourse import mybir
    from concourse.bass2jax import (_bass_exec_p, install_neuronx_cc_hook,
                                    partition_id_tensor)
    install_neuronx_cc_hook()

    partition_name = nc.partition_id_tensor.name if nc.partition_id_tensor else None
    in_names, out_names, out_avals = [], [], []
    for alloc in nc.m.functions[0].allocations:
        if not isinstance(alloc, mybir.MemoryLocationSet):
            continue
        name = alloc.memorylocations[0].name
        if alloc.kind == "ExternalInput":
            if name != partition_name:
                in_names.append(name)
        elif alloc.kind == "ExternalOutput":
            out_names.append(name)
            out_avals.append(jax.core.ShapedArray(tuple(alloc.tensor_shape),
                                                  mybir.dt.np(alloc.dtype)))
    assert out_names == ["out"], out_names
    expect = [n for n in IN_NAMES if n != "out"]
    assert in_names == expect, (in_names, expect)
    all_names = in_names + out_names + ([partition_name] if partition_name else [])
    n_params = len(in_names)

    def _body(*args):
        operands = list(args)
        if partition_name is not None:
            operands.append(partition_id_tensor())
        return tuple(_bass_exec_p.bind(
            *operands, out_avals=tuple(out_avals), in_names=tuple(all_names),
            out_names=tuple(out_names), lowering_input_output_aliases=(),
            sim_require_finite=True, sim_require_nnan=True, nc=nc))

    donate = tuple(range(n_params, n_params + len(out_names)))
    sharded = jax.jit(
        shard_map(_body, mesh=mesh,
                  in_specs=(PartitionSpec("core"),) * (n_params + len(out_names)),
                  out_specs=(PartitionSpec("core"),) * len(out_names), check_rep=False),
        donate_argnums=donate, keep_unused=True)

    # reorder dev_args to (params..., out_zero)
    by_name = dict(zip(param_names, dev_args))
    call_args = [by_name[n] for n in in_names] + [by_name["out"]]
    compiled = sharded.lower(*call_args).compile()
    t4 = time.perf_counter()
    print(f"[kernel] jit compile: {t4 - t3:.2f}s", file=sys.stderr, flush=True)

    out_arrs = compiled(*call_args)
    jax.block_until_ready(out_arrs)
    t5 = time.perf_counter()
    print(f"[kernel] exec: {t5 - t4:.2f}s", file=sys.stderr, flush=True)
    kernel.last_exec_ns = None

    full = np.asarray(out_arrs[0]).reshape(n_cores, SHARD_P, D)
    out = np.zeros((N, D), np.float32)
    for c in range(n_cores):
        out[c * SHARD:(c + 1) * SHARD] = full[c][:SHARD]
    print(f"[kernel] post: {time.perf_counter() - t5:.2f}s", file=sys.stderr, flush=True)
    return out


# revision 12
# speedup vs baseline: 3.3726x; 1.4986x over previous
"""GAT-KH (2-layer, 3-hop, 8-head GAT, N=50k, E=300k/hop) on 8 TRN2 cores.

Distribution: dst-sharded edges, sharded gather-table build + AllGather.
- Nodes renumbered into 8 padded shards of 6400 (NP=51200). Core c owns dst
  shard c (tiles of 128 dsts, 50 tiles).
- Per layer: core c computes h^T for its own shard (SBUF-resident), builds
  its shard's slice of the 3 per-hop gather tables T_k[n] = [hp(256) |
  alpha_src(8)] bf16 (768B row stride), then AllGathers each table so every
  core holds all NP rows.
- Edge phase: per dst tile, edges chunked by 128 (split src<32768 for int16
  dma_gather), gathered rows -> softmax numerators w=exp(leaky(as+ad)+mb)
  -> one-hot P matmuls scatter [w*hp | w] into PSUM -> normalize -> dec_w
  -> leaky -> decay-accumulate.
- Layer end: LayerNorm+residual; h1^T transposed back into SBUF for layer-2
  tables. Output: per-core h shard (bf16), host-concatenated.
- Runner: inputs are device_put on a background thread (sharded over the 8
  cores) while the bass build + NEFF compile run, then executed via PJRT.
"""

import numpy as np
import ml_dtypes
from contextlib import ExitStack

N = 50000
E = 300000
HOPS = 3
LAYERS = 2
HEADS = 8
D = 256
DH = 32
NCORES = 8
SHARD = 6250
SHARD_P = 6400
TILES = SHARD_P // 128            # 50
NP = NCORES * SHARD_P             # 51200
ROWB = 384                        # table row length in bf16 elems (768B)
TCOLS = 272                       # wcat cols: [W(256) | a_src(8) | a_dst(8)]
TSTORE = 264                      # table cols actually stored: hp + alpha_src
LH_SPLIT = 32768
DECAY = [float(np.exp(-0.5 * k)) for k in range(HOPS)]
SLOPE_ACT = 0.01
SLOPE_ATT = 0.2
LN_EPS = 1e-5
NEG_BIAS = -30000.0
TPG = 3                           # dst tiles per gather group
GT = 10                           # table chunks per staging group (50%GT==0)
LNCH = 10                         # layernorm tiles per chunk (50%LNCH==0)
BF16 = ml_dtypes.bfloat16


def _pack_idx16(idx):
    """int16 idx list -> [16, ceil(n/16)] wrapped in 16 partitions."""
    n = len(idx)
    n16 = max(1, (n + 15) // 16)
    pad = np.zeros(n16 * 16, np.int16)
    pad[:n] = idx
    return pad.reshape(n16, 16).T.copy()


def _host_prep(inputs):
    x = np.asarray(inputs["x"], np.float32)
    ei = np.asarray(inputs["edge_index_k_hops"])
    lin1_w = np.asarray(inputs["lin1_w"], np.float32)
    gat_w = np.asarray(inputs["gat_w"], np.float32)
    a_src = np.asarray(inputs["gat_att_src"], np.float32)
    a_dst = np.asarray(inputs["gat_att_dst"], np.float32)
    dec_w = np.asarray(inputs["dec_w"], np.float32)

    wcat = np.zeros((LAYERS, HOPS, D, TCOLS), np.float32)
    Wh = gat_w.reshape(LAYERS, HOPS, D, HEADS, DH)
    wcat[:, :, :, :D] = gat_w
    wcat[:, :, :, D:D + HEADS] = np.einsum("lkdhc,lkhc->lkdh", Wh, a_src)
    wcat[:, :, :, D + HEADS:] = np.einsum("lkdhc,lkhc->lkdh", Wh, a_dst)

    xT = np.zeros((D, NP), np.float32)
    xT.reshape(D, NCORES, SHARD_P)[:, :, :SHARD] = \
        x.reshape(NCORES, SHARD, D).transpose(2, 0, 1)
    xT_bf = xT.astype(BF16)
    xTs = [xT_bf[:, c * SHARD_P:(c + 1) * SHARD_P].reshape(2, 128, SHARD_P)
           for c in range(NCORES)]

    # ---- edge routing (vectorized) ----
    cnts = np.zeros((HOPS, NCORES, TILES, 2), np.int64)
    hop_sorted = []
    for k in range(HOPS):
        src = ei[k, 0].astype(np.int64)
        dst = ei[k, 1].astype(np.int64)
        ps = (src // SHARD) * SHARD_P + (src % SHARD)
        core = dst // SHARD
        dl = dst % SHARD
        tl = dl // 128
        dloc = dl % 128
        side = (ps >= LH_SPLIT).astype(np.int64)
        keyfull = (core * TILES + tl) * 2 + side
        order = np.argsort(keyfull, kind="stable")
        cnt = np.bincount(keyfull, minlength=NCORES * TILES * 2)
        cnts[k] = cnt.reshape(NCORES, TILES, 2)
        hop_sorted.append((keyfull[order], ps[order], dloc[order], side[order], cnt))
    nch = np.maximum(1, np.ceil(cnts.max(axis=1) / 128.0)).astype(np.int64)

    core_data = [[] for _ in range(NCORES)]
    for k in range(HOPS):
        kf_s, ps_s, dloc_s, side_s, cnt = hop_sorted[k]
        cap_flat = (nch[k].reshape(-1) * 128).astype(np.int64)   # per (t,side)
        bucket_off = np.zeros(TILES * 2, np.int64)
        bucket_off[1:] = np.cumsum(cap_flat)[:-1]
        total_cap = int(cap_flat.sum())                          # NCH[k]*128
        starts = np.zeros(NCORES * TILES * 2, np.int64)
        starts[1:] = np.cumsum(cnt)[:-1]
        rank = np.arange(E, dtype=np.int64) - starts[kf_s]
        dest = (kf_s // (TILES * 2)) * total_cap + bucket_off[kf_s % (TILES * 2)] + rank

        idxA = np.zeros(NCORES * total_cap, np.int64)
        dvA = np.zeros(NCORES * total_cap, np.int64)
        mbA = np.full(NCORES * total_cap, NEG_BIAS, np.float32)
        idxA[dest] = ps_s - side_s * LH_SPLIT
        dvA[dest] = dloc_s
        mbA[dest] = 0.0

        Lsel = np.concatenate([np.arange(bucket_off[2 * t], bucket_off[2 * t] + cap_flat[2 * t])
                               for t in range(TILES)])
        Hsel = np.concatenate([np.arange(bucket_off[2 * t + 1], bucket_off[2 * t + 1] + cap_flat[2 * t + 1])
                               for t in range(TILES)])
        for c in range(NCORES):
            blk = slice(c * total_cap, (c + 1) * total_cap)
            idxc = idxA[blk]
            dvc = dvA[blk].astype(np.float32)
            nchk = total_cap // 128
            core_data[c].append({
                "Lidx": _pack_idx16(idxc[Lsel].astype(np.int16)),
                "Hidx": _pack_idx16(idxc[Hsel].astype(np.int16)),
                "dstloc": dvc.reshape(nchk, 128).T.astype(BF16),
                "dstrow": dvc.astype(BF16).reshape(1, -1),
                "maskb": mbA[blk].reshape(nchk, 128).T.astype(BF16),
            })

    iota = np.tile(np.arange(128, dtype=np.float32)[None, :], (128, 1)).astype(BF16)
    iotac = np.ascontiguousarray(np.arange(128, dtype=np.float32)[:, None])
    return {
        "iotac": iotac,
        "wcat": wcat.astype(BF16), "xTs": xTs,
        "lin1": lin1_w.astype(BF16), "dec": dec_w.astype(BF16),
        "core_data": core_data, "nch": nch, "iota": iota,
    }


def _build(prep, stage=5, edgelvl=5, no_lrelu=False, no_v4d=False, no_lnb=False):
    from concourse import bass, mybir, tile, library_config
    from concourse.bass import AP
    from concourse.masks import make_identity
    import concourse.bacc as bacc

    nch = prep["nch"]
    cd0 = prep["core_data"][0]
    NCH = [cd0[k]["dstloc"].shape[1] for k in range(HOPS)]
    N16L = [cd0[k]["Lidx"].shape[1] for k in range(HOPS)]
    N16H = [cd0[k]["Hidx"].shape[1] for k in range(HOPS)]

    fp32 = mybir.dt.float32
    bf16 = mybir.dt.bfloat16
    i16 = mybir.dt.int16
    AF = mybir.ActivationFunctionType
    OP = mybir.AluOpType

    nc = bacc.Bacc("TRN2", target_bir_lowering=False, debug=False,
                   num_devices=NCORES)

    d_xTs = nc.dram_tensor("xTs", [2, 128, SHARD_P], bf16, kind="ExternalInput")
    d_lin1 = nc.dram_tensor("lin1", [D, D], bf16, kind="ExternalInput")
    d_wcat = nc.dram_tensor("wcat", [LAYERS, HOPS, D, TCOLS], bf16, kind="ExternalInput")
    d_dec = nc.dram_tensor("dec", [LAYERS, HOPS, D, D], bf16, kind="ExternalInput")
    d_iota = nc.dram_tensor("iota", [128, 128], bf16, kind="ExternalInput")
    d_gidx = [(nc.dram_tensor(f"gidxL{k}", [16, N16L[k]], i16, kind="ExternalInput"),
               nc.dram_tensor(f"gidxH{k}", [16, N16H[k]], i16, kind="ExternalInput"))
              for k in range(HOPS)]
    d_dstloc = [nc.dram_tensor(f"dstloc{k}", [128, NCH[k]], bf16, kind="ExternalInput") for k in range(HOPS)]
    d_dstrow = [nc.dram_tensor(f"dstrow{k}", [1, NCH[k] * 128], bf16, kind="ExternalInput") for k in range(HOPS)]
    d_iotac = nc.dram_tensor("iotac", [128, 1], fp32, kind="ExternalInput")
    d_maskb = [nc.dram_tensor(f"maskb{k}", [128, NCH[k]], bf16, kind="ExternalInput") for k in range(HOPS)]
    d_out = nc.dram_tensor("out", [SHARD_P, D], bf16, kind="ExternalOutput")

    d_tabin = [nc.dram_tensor(f"tabin{k}", [SHARD_P, ROWB], bf16, kind="Internal")
               for k in range(HOPS)]
    d_tab = [nc.dram_tensor(f"tab{k}", [NP, ROWB], bf16, kind="Internal",
                            addr_space="Shared") for k in range(HOPS)]

    # per-hop static chunk bookkeeping
    col0 = []   # [k][t] first chunk col of tile t
    lcum = []   # [k][t] first L-chunk index (global, within hop) of tile t
    hcum = []
    for k in range(HOPS):
        c0, lc, hc = [], [], []
        a = b = g = 0
        for t in range(TILES):
            c0.append(g)
            lc.append(a)
            hc.append(b)
            a += int(nch[k, t, 0])
            b += int(nch[k, t, 1])
            g += int(nch[k, t, 0] + nch[k, t, 1])
        col0.append(c0)
        lcum.append(lc)
        hcum.append(hc)

    groups = [list(range(g0, min(g0 + TPG, TILES))) for g0 in range(0, TILES, TPG)]

    def bcast_mid(ap, n):
        """[128, M] AP -> [128, n, M] with 0-step middle dim."""
        return AP(ap.tensor, ap.offset, [list(ap.ap[0]), [0, n], list(ap.ap[1])])

    with tile.TileContext(nc) as tc:
        with ExitStack() as ctx:
            persist = ctx.enter_context(tc.tile_pool(name="persist", bufs=1))
            nc.gpsimd.load_library(library_config.mlp)

            sb_iota = persist.tile((128, 128), bf16)
            nc.sync.dma_start(sb_iota[:], d_iota[:, :])
            sb_ident = persist.tile((128, 128), bf16)
            make_identity(nc, sb_ident[:])
            sb_lin1 = persist.tile((128, 2, D), bf16)
            for kc in range(2):
                nc.sync.dma_start(sb_lin1[:, kc, :], d_lin1[kc * 128:(kc + 1) * 128, :])
            sb_wcat = persist.tile((128, LAYERS, HOPS, 2, TCOLS), bf16)
            sb_dec = persist.tile((128, LAYERS, HOPS, 2, D), bf16)
            for l in range(LAYERS):
                for k in range(HOPS):
                    for kc in range(2):
                        nc.sync.dma_start(sb_wcat[:, l, k, kc, :],
                                          d_wcat[l, k, kc * 128:(kc + 1) * 128, :])
                        nc.sync.dma_start(sb_dec[:, l, k, kc, :],
                                          d_dec[l, k, kc * 128:(kc + 1) * 128, :])
            sb_acc = persist.tile((128, TILES, D), fp32)
            sb_res = persist.tile((128, TILES, D), bf16)
            sb_iotac = persist.tile((128, 1), fp32)
            nc.sync.dma_start(sb_iotac[:], d_iotac[:, :])
            sb_ad = persist.tile((128, TILES, HOPS, 8), bf16)
            sb_hT = persist.tile((128, 2, SHARD_P), bf16)

            # ---------- phase 0: h0^T for own shard (SBUF-resident) ----------
            with tc.tile_pool(name="p0", bufs=1) as p0x, \
                 tc.tile_pool(name="p0ps", bufs=4, space="PSUM") as p0ps:
                sb_xT = p0x.tile((128, 2, SHARD_P), bf16)
                for kc in range(2):
                    nc.sync.dma_start(sb_xT[:, kc, :], d_xTs[kc, :, :])
                SUP = 512
                for s0 in range(0, SHARD_P, SUP):
                    w = min(SUP, SHARD_P - s0)
                    for mc in range(2):
                        ps = p0ps.tile((128, SUP), fp32, tag="ps")
                        for kc in range(2):
                            nc.tensor.matmul(ps[:, 0:w],
                                             lhsT=sb_lin1[:, kc, mc * 128:(mc + 1) * 128],
                                             rhs=sb_xT[:, kc, s0:s0 + w],
                                             start=(kc == 0), stop=(kc == 1))
                        if no_lrelu:
                            r = p0x.tile((128, SUP), fp32, tag="r", name=f"p0r{s0}_{mc}")
                            nc.scalar.activation(r[:, 0:w], ps[:, 0:w], AF.Relu,
                                                 scale=1.0 - SLOPE_ACT)
                            h0 = p0x.tile((128, SUP), fp32, tag="h0", name=f"p0h{s0}_{mc}")
                            nc.vector.tensor_scalar(h0[:, 0:w], ps[:, 0:w], SLOPE_ACT, None, OP.mult)
                            nc.vector.tensor_add(sb_hT[:, mc, s0:s0 + w], h0[:, 0:w], r[:, 0:w])
                        else:
                            nc.scalar.activation(sb_hT[:, mc, s0:s0 + w], ps[:, 0:w],
                                                 AF.Lrelu, alpha=SLOPE_ACT)

            # ---------- phase 0b: residual h0 rows from sb_hT ----------
            with tc.tile_pool(name="p0bps", bufs=4, space="PSUM") as p0bps:
                for t in range(TILES):
                    for kc in range(2):
                        pt = p0bps.tile((128, 128), bf16, tag="pt")
                        nc.tensor.transpose(pt[:], sb_hT[:, kc, t * 128:(t + 1) * 128],
                                            sb_ident[:])
                        nc.vector.tensor_copy(sb_res[:, t, kc * 128:(kc + 1) * 128], pt[:])

            # ================= layers =================
            for l in range(LAYERS if stage >= 5 else 1):
                # ---- alpha_dst per own-shard tile ----
                with tc.tile_pool(name=f"adps{l}", bufs=4, space="PSUM") as padps:
                    for t in range(TILES):
                        psa = padps.tile((128, HOPS * 8), fp32, tag="psa")
                        for k in range(HOPS):
                            for kc in range(2):
                                nc.tensor.matmul(psa[:, k * 8:(k + 1) * 8],
                                                 lhsT=sb_hT[:, kc, t * 128:(t + 1) * 128],
                                                 rhs=sb_wcat[:, l, k, kc, D + HEADS:D + 2 * HEADS],
                                                 start=(kc == 0), stop=(kc == 1),
                                                 skip_group_check=True)
                        nc.vector.tensor_copy(
                            sb_ad[:, t, :, :].rearrange("p k h -> p (k h)"), psa[:])

                # ---- own-shard tables for all 3 hops, then AllGather ----
                if stage < 1:
                    break
                with tc.tile_pool(name=f"tb{l}", bufs=2) as ptb, \
                     tc.tile_pool(name=f"tbps{l}", bufs=2, space="PSUM") as ptbps:
                    for g in range(TILES // GT):
                        n0 = g * GT * 128
                        stg = [ptb.tile((128, GT, TSTORE), bf16, tag=f"st{k}",
                                        name=f"stg{l}_{g}_{k}") for k in range(HOPS)]
                        for ci in range(GT):
                            cs = n0 + ci * 128
                            pks = [ptbps.tile((128, TSTORE), fp32, tag=f"tp{k}",
                                              name=f"pks{l}_{g}_{ci}_{k}") for k in range(HOPS)]
                            for kc in range(2):
                                for k in range(HOPS):
                                    nc.tensor.matmul(pks[k][:],
                                                     lhsT=sb_hT[:, kc, cs:cs + 128],
                                                     rhs=sb_wcat[:, l, k, kc, 0:TSTORE],
                                                     start=(kc == 0), stop=(kc == 1),
                                                     skip_group_check=True)
                            for k in range(HOPS):
                                if k == 1:
                                    nc.scalar.copy(stg[k][:, ci, :], pks[k][:])
                                else:
                                    nc.vector.tensor_copy(stg[k][:, ci, :], pks[k][:])
                        for k in range(HOPS):
                            nc.sync.dma_start(
                                d_tabin[k][n0:n0 + GT * 128, 0:TSTORE]
                                .rearrange("(g p) c -> p g c", p=128),
                                stg[k][:])
                for k in range(HOPS):
                    nc.gpsimd.collective_compute(
                        "AllGather", mybir.AluOpType.bypass,
                        replica_groups=[list(range(NCORES))],
                        ins=[d_tabin[k][:, :]],
                        outs=[d_tab[k][:, :]],
                    )

                # ---- edge phase: 3 hops ----
                nhops = HOPS if stage >= 3 else (1 if stage >= 2 else 0)
                for k in range(nhops):
                    dl_k = d_gidx[k]
                    with tc.tile_pool(name=f"eg{l}{k}", bufs=2) as peg, \
                         tc.tile_pool(name=f"egix{l}{k}", bufs=1) as pix, \
                         tc.tile_pool(name=f"egsc{l}{k}", bufs=2, space="PSUM") as psc, \
                         tc.tile_pool(name=f"egtr{l}{k}", bufs=2, space="PSUM") as ptr, \
                         tc.tile_pool(name=f"egdc{l}{k}", bufs=2, space="PSUM") as pdc:
                        # hop-wide gather indices: load 16-partition packed
                        # form once, replicate to all 8 partition groups.
                        ixLf = pix.tile((128, N16L[k]), i16)
                        ixHf = pix.tile((128, N16H[k]), i16)
                        for (ixf, dsrc) in ((ixLf, dl_k[0]), (ixHf, dl_k[1])):
                            nc.sync.dma_start(ixf[0:16, :], dsrc[:, :])
                            for gpi in range(1, 8):
                                nc.sync.dma_start(ixf[16 * gpi:16 * (gpi + 1), :],
                                                  ixf[0:16, :])
                        for grp in groups:
                            t0 = grp[0]
                            gnL = int(sum(nch[k, t, 0] for t in grp))
                            gnH = int(sum(nch[k, t, 1] for t in grp))
                            gnc = int(sum(nch[k, t, 0] + nch[k, t, 1] for t in grp))
                            oL = lcum[k][t0]
                            oH = hcum[k][t0]
                            oc = col0[k][t0]

                            GSL = 28  # chunks per dma_gather
                            gbL = peg.tile((128, gnL, ROWB), bf16, tag="gbL")
                            for off in range(0, gnL, GSL):
                                cnt = min(GSL, gnL - off)
                                nc.gpsimd.dma_gather(
                                    out_ap=gbL[:, off:off + cnt, :],
                                    in_ap=d_tab[k][0:LH_SPLIT, :],
                                    idxs_ap=ixLf[:, (oL + off) * 8:(oL + off + cnt) * 8],
                                    num_idxs=cnt * 128,
                                    num_idxs_reg=cnt * 128, elem_size=ROWB,
                                    single_packet=False)
                            gbH = peg.tile((128, gnH, ROWB), bf16, tag="gbH")
                            for off in range(0, gnH, GSL):
                                cnt = min(GSL, gnH - off)
                                nc.gpsimd.dma_gather(
                                    out_ap=gbH[:, off:off + cnt, :],
                                    in_ap=d_tab[k][LH_SPLIT:NP, :],
                                    idxs_ap=ixHf[:, (oH + off) * 8:(oH + off + cnt) * 8],
                                    num_idxs=cnt * 128,
                                    num_idxs_reg=cnt * 128, elem_size=ROWB,
                                    single_packet=False)
                            dls = peg.tile((128, gnc), bf16, tag="dls")
                            nc.sync.dma_start(dls[:], d_dstloc[k][:, oc:oc + gnc])
                            mbs = peg.tile((128, gnc), bf16, tag="mbs")
                            nc.sync.dma_start(mbs[:], d_maskb[k][:, oc:oc + gnc])
                            drow = peg.tile((128, gnc * 128), bf16, tag="drow")
                            nc.sync.dma_start(drow[0:1, :], d_dstrow[k][0:1, oc * 128:(oc + gnc) * 128])
                            nc.gpsimd.partition_broadcast(drow[:], drow[0:1, :])

                            for t in grp:
                                if edgelvl < 2:
                                    continue
                                nL = int(nch[k, t, 0])
                                nH = int(nch[k, t, 1])
                                nT = nL + nH
                                lj = lcum[k][t] - oL
                                hj = hcum[k][t] - oH
                                cj = col0[k][t] - oc

                                V = peg.tile((128, nT, 264), bf16, tag="V")
                                e0 = peg.tile((128, nT, 8), fp32, tag="e0")
                                e1 = peg.tile((128, nT, 8), fp32, tag="e1")
                                PT = peg.tile((128, nT * 128), bf16, tag="PT")
                                nc.vector.tensor_scalar(
                                    PT[:], drow[:, cj * 128:(cj + nT) * 128],
                                    sb_iotac[:, 0:1], None, OP.is_equal)
                                pad_ps = psc.tile((128, nT, 8), fp32, tag="pad_ps")
                                for j in range(nT):
                                    nc.tensor.matmul(pad_ps[:, j, :],
                                                     lhsT=PT[:, j * 128:(j + 1) * 128],
                                                     rhs=sb_ad[:, t, k, :],
                                                     start=True, stop=True,
                                                     skip_group_check=True)
                                nc.vector.tensor_tensor(
                                    e0[:, 0:nL, :], gbL[:, lj:lj + nL, 256:264],
                                    pad_ps[:, 0:nL, :], op=OP.add)
                                nc.vector.tensor_tensor(
                                    e0[:, nL:nT, :], gbH[:, hj:hj + nH, 256:264],
                                    pad_ps[:, nL:nT, :], op=OP.add)
                                # NB: Lrelu's alpha is baked into the ACT
                                # table once per kernel; a second distinct
                                # alpha is silently ignored, so the 0.2
                                # attention slope stays on the vector engine.
                                nc.vector.tensor_scalar(e1[:], e0[:], SLOPE_ATT, None, OP.mult)
                                nc.vector.tensor_tensor(e1[:], e0[:], e1[:], op=OP.max)
                                nc.vector.tensor_tensor(
                                    e1[:], e1[:],
                                    mbs[:, cj:cj + nT].to_broadcast((128, nT, 8)),
                                    op=OP.add)
                                nc.scalar.activation(V[:, :, 256:264], e1[:], AF.Exp)
                                if edgelvl < 3:
                                    continue
                                if no_v4d:
                                    for j in range(nT):
                                        srcap = (gbL[:, lj + j, 0:256] if j < nL
                                                 else gbH[:, hj + (j - nL), 0:256])
                                        nc.vector.tensor_tensor(
                                            V[:, j, 0:256].rearrange("p (h c) -> p h c", h=HEADS),
                                            srcap.rearrange("p (h c) -> p h c", h=HEADS),
                                            V[:, j, 256:264].to_broadcast((128, HEADS, DH)),
                                            op=OP.mult)
                                else:
                                    if nL > 0:
                                        nc.vector.tensor_tensor(
                                            V[:, 0:nL, 0:256].rearrange("p j (h c) -> p j h c", h=HEADS),
                                            gbL[:, lj:lj + nL, 0:256].rearrange("p j (h c) -> p j h c", h=HEADS),
                                            V[:, 0:nL, 256:264].to_broadcast((128, nL, HEADS, DH)),
                                            op=OP.mult)
                                    if nH > 0:
                                        nc.vector.tensor_tensor(
                                            V[:, nL:nT, 0:256].rearrange("p j (h c) -> p j h c", h=HEADS),
                                            gbH[:, hj:hj + nH, 0:256].rearrange("p j (h c) -> p j h c", h=HEADS),
                                            V[:, nL:nT, 256:264].to_broadcast((128, nH, HEADS, DH)),
                                            op=OP.mult)
                                if edgelvl < 4:
                                    continue
                                P = peg.tile((128, nT, 128), bf16, tag="P")
                                nc.vector.tensor_tensor(
                                    P[:], dls[:, cj:cj + nT].to_broadcast((128, nT, 128)),
                                    bcast_mid(sb_iota[:], nT), op=OP.is_equal)
                                ps = psc.tile((128, 264), fp32, tag="ps")
                                for j in range(nT):
                                    nc.tensor.matmul(ps[:], lhsT=P[:, j, :], rhs=V[:, j, :],
                                                     start=(j == 0), stop=(j == nT - 1))
                                den = peg.tile((128, 8), fp32, tag="den")
                                nc.vector.tensor_scalar(den[:], ps[:, 256:264], 1e-16, None, OP.add)
                                rec = peg.tile((128, 8), fp32, tag="rec")
                                nc.vector.reciprocal(rec[:], den[:])
                                Gt = peg.tile((128, D), bf16, tag="Gt")
                                nc.vector.tensor_tensor(
                                    Gt[:].rearrange("p (h c) -> p h c", h=HEADS),
                                    ps[:, 0:256].rearrange("p (h c) -> p h c", h=HEADS),
                                    rec[:].to_broadcast((128, HEADS, DH)), op=OP.mult)
                                if edgelvl < 5:
                                    continue
                                GtT = peg.tile((128, 2, 128), bf16, tag="GtT")
                                for kc in range(2):
                                    pt = ptr.tile((128, 128), bf16, tag="pt")
                                    nc.tensor.transpose(pt[:], Gt[:, kc * 128:(kc + 1) * 128],
                                                        sb_ident[:])
                                    nc.vector.tensor_copy(GtT[:, kc, :], pt[:])
                                pd = pdc.tile((128, D), fp32, tag="pd")
                                for kc in range(2):
                                    nc.tensor.matmul(pd[:], lhsT=GtT[:, kc, :],
                                                     rhs=sb_dec[:, l, k, kc, :],
                                                     start=(kc == 0), stop=(kc == 1))
                                if no_lrelu:
                                    u1 = peg.tile((128, D), fp32, tag="u1")
                                    nc.vector.tensor_scalar(u1[:], pd[:], DECAY[k], None, OP.mult)
                                    u2 = peg.tile((128, D), fp32, tag="u2")
                                    nc.scalar.activation(u2[:], pd[:], AF.Copy,
                                                         scale=DECAY[k] * SLOPE_ACT)
                                    if k == 0:
                                        nc.vector.tensor_tensor(sb_acc[:, t, :], u1[:], u2[:], op=OP.max)
                                    else:
                                        u3 = peg.tile((128, D), fp32, tag="u3")
                                        nc.vector.tensor_tensor(u3[:], u1[:], u2[:], op=OP.max)
                                        nc.vector.tensor_add(sb_acc[:, t, :], sb_acc[:, t, :], u3[:])
                                elif k == 0:
                                    nc.scalar.activation(sb_acc[:, t, :], pd[:],
                                                         AF.Lrelu, scale=DECAY[k],
                                                         alpha=SLOPE_ACT)
                                else:
                                    u3 = peg.tile((128, D), fp32, tag="u3")
                                    nc.scalar.activation(u3[:], pd[:], AF.Lrelu,
                                                         scale=DECAY[k], alpha=SLOPE_ACT)
                                    nc.vector.tensor_add(sb_acc[:, t, :], sb_acc[:, t, :], u3[:])

                # ---- layer norm + residual (+ next-layer hT / output) ----
                if stage < 3 or edgelvl < 5:
                    break
                with tc.tile_pool(name=f"ln{l}", bufs=2) as pln, \
                     tc.tile_pool(name=f"lnps{l}", bufs=2, space="PSUM") as plnps:
                    if no_lnb:
                        for t in range(TILES):
                            acc = sb_acc[:, t, :]
                            mu = pln.tile((128, 1), fp32, tag="mu1")
                            nc.vector.reduce_sum(mu[:], acc, axis=mybir.AxisListType.X)
                            nc.vector.tensor_scalar(mu[:], mu[:], 1.0 / D, None, OP.mult)
                            xc = pln.tile((128, D), fp32, tag="xc1")
                            nc.vector.tensor_scalar(xc[:], acc, mu[:, 0:1], None, OP.subtract)
                            sq = pln.tile((128, D), fp32, tag="sq1")
                            nc.vector.tensor_tensor(sq[:], xc[:], xc[:], op=OP.mult)
                            var = pln.tile((128, 1), fp32, tag="var1")
                            nc.vector.reduce_sum(var[:], sq[:], axis=mybir.AxisListType.X)
                            nc.vector.tensor_scalar(var[:], var[:], 1.0 / D, LN_EPS,
                                                    OP.mult, OP.add)
                            sd = pln.tile((128, 1), fp32, tag="sd1")
                            nc.scalar.activation(sd[:], var[:], AF.Sqrt)
                            nc.vector.reciprocal(sd[:], sd[:])
                            hn = pln.tile((128, D), fp32, tag="hn1")
                            nc.vector.tensor_tensor(hn[:], xc[:], sd[:].to_broadcast((128, D)),
                                                    op=OP.mult)
                            nc.vector.tensor_add(hn[:], hn[:], sb_res[:, t, :])
                            if l == 0:
                                nc.vector.tensor_copy(sb_res[:, t, :], hn[:])
                                for kc in range(2):
                                    pt = plnps.tile((128, 128), bf16, tag="pt")
                                    nc.tensor.transpose(pt[:], sb_res[:, t, kc * 128:(kc + 1) * 128],
                                                        sb_ident[:])
                                    nc.vector.tensor_copy(sb_hT[:, kc, t * 128:(t + 1) * 128], pt[:])
                            else:
                                hb = pln.tile((128, D), bf16, tag="hb1")
                                nc.vector.tensor_copy(hb[:], hn[:])
                                nc.sync.dma_start(d_out[t * 128:(t + 1) * 128, :], hb[:])
                        continue
                    for j0 in range(0, TILES, LNCH):
                        A = sb_acc[:, j0:j0 + LNCH, :]
                        mu = pln.tile((128, LNCH), fp32, tag="mu")
                        nc.vector.reduce_sum(mu[:], A, axis=mybir.AxisListType.X)
                        nc.vector.tensor_scalar(mu[:], mu[:], 1.0 / D, None, OP.mult)
                        nc.vector.tensor_tensor(
                            A, A, mu[:].to_broadcast((128, LNCH, D)), op=OP.subtract)
                        sq = pln.tile((128, LNCH, D), fp32, tag="sq")
                        nc.vector.tensor_tensor(sq[:], A, A, op=OP.mult)
                        var = pln.tile((128, LNCH), fp32, tag="var")
                        nc.vector.reduce_sum(var[:], sq[:], axis=mybir.AxisListType.X)
                        nc.vector.tensor_scalar(var[:], var[:], 1.0 / D, LN_EPS,
                                                OP.mult, OP.add)
                        sd = pln.tile((128, LNCH), fp32, tag="sd")
                        nc.scalar.activation(sd[:], var[:], AF.Sqrt)
                        nc.vector.reciprocal(sd[:], sd[:])
                        hn = pln.tile((128, LNCH, D), fp32, tag="hn")
                        nc.vector.tensor_tensor(
                            hn[:], A, sd[:].to_broadcast((128, LNCH, D)), op=OP.mult)
                        nc.vector.tensor_add(hn[:], hn[:], sb_res[:, j0:j0 + LNCH, :])
                        if l == 0:
                            nc.vector.tensor_copy(sb_res[:, j0:j0 + LNCH, :], hn[:])
                            for ci in range(LNCH):
                                t = j0 + ci
                                for kc in range(2):
                                    pt = plnps.tile((128, 128), bf16, tag="pt")
                                    nc.tensor.transpose(
                                        pt[:], sb_res[:, t, kc * 128:(kc + 1) * 128],
                                        sb_ident[:])
                                    nc.vector.tensor_copy(
                                        sb_hT[:, kc, t * 128:(t + 1) * 128], pt[:])
                        else:
                            hb = pln.tile((128, LNCH, D), bf16, tag="hb")
                            nc.vector.tensor_copy(hb[:], hn[:])
                            nc.sync.dma_start(
                                d_out[j0 * 128:(j0 + LNCH) * 128, :]
                                .rearrange("(t p) c -> p t c", p=128),
                                hb[:])

            if stage < 5:
                with tc.tile_pool(name="dbg", bufs=2) as pdbg:
                    for t in range(TILES):
                        z = pdbg.tile((128, D), bf16, tag="z")
                        nc.vector.tensor_copy(z[:], sb_res[:, t, :])
                        nc.sync.dma_start(d_out[t * 128:(t + 1) * 128, :], z[:])

    nc.compile()
    return nc


IN_NAMES = (["xTs", "lin1", "wcat", "dec", "iota"]
            + [n for k in range(HOPS) for n in (f"gidxL{k}", f"gidxH{k}")]
            + [f"dstloc{k}" for k in range(HOPS)]
            + [f"dstrow{k}" for k in range(HOPS)]
            + ["iotac"] + [f"maskb{k}" for k in range(HOPS)]
            + ["out"])


def kernel(**inputs):
    import os, sys, time, threading
    import jax
    from jax.sharding import Mesh, PartitionSpec, NamedSharding

    t0 = time.perf_counter()
    prep = _host_prep(inputs)
    t1 = time.perf_counter()
    print(f"[kernel] host_prep: {t1 - t0:.2f}s", file=sys.stderr, flush=True)

    n_cores = NCORES
    devices = jax.devices()[:n_cores]
    mesh = Mesh(np.asarray(devices), ("core",))
    sharding = NamedSharding(mesh, PartitionSpec("core"))

    def core_map(c):
        m = {"xTs": np.ascontiguousarray(prep["xTs"][c]),
             "lin1": prep["lin1"], "wcat": prep["wcat"], "dec": prep["dec"],
             "iota": prep["iota"], "iotac": prep["iotac"]}
        for k in range(HOPS):
            cd = prep["core_data"][c][k]
            m[f"gidxL{k}"] = cd["Lidx"]
            m[f"gidxH{k}"] = cd["Hidx"]
            m[f"dstloc{k}"] = cd["dstloc"]
            m[f"dstrow{k}"] = cd["dstrow"]
            m[f"maskb{k}"] = cd["maskb"]
        m["out"] = np.zeros((SHARD_P, D), BF16)
        return m

    maps = [core_map(c) for c in range(n_cores)]
    dev = {}
    specs = {}
    for name in IN_NAMES:
        a0 = maps[0][name]
        specs[name] = jax.ShapeDtypeStruct((n_cores * a0.shape[0],) + a0.shape[1:],
                                           a0.dtype, sharding=sharding)

    def _upload():
        for name in IN_NAMES:
            cat = np.concatenate([maps[c][name] for c in range(n_cores)], axis=0)
            dev[name] = jax.device_put(cat, sharding)

    up_th = threading.Thread(target=_upload)
    up_th.start()

    nc = _build(prep, stage=int(os.environ.get("GAT_STAGE", "5")),
                edgelvl=int(os.environ.get("GAT_EDGELVL", "5")),
                no_lrelu=bool(int(os.environ.get("GAT_NOLRELU", "0"))),
                no_v4d=bool(int(os.environ.get("GAT_NOV4D", "0"))),
                no_lnb=bool(int(os.environ.get("GAT_NOLNB", "0"))))
    t2 = time.perf_counter()
    print(f"[kernel] build+compile: {t2 - t1:.2f}s", file=sys.stderr, flush=True)

    # ---- PJRT execution (mirrors run_bass_via_pjrt, with device inputs) ----
    from jax.experimental.shard_map import shard_map
    from concourse import mybir
    from concourse.bass2jax import (_bass_exec_p, install_neuronx_cc_hook,
                                    partition_id_tensor)
    install_neuronx_cc_hook()

    partition_name = nc.partition_id_tensor.name if nc.partition_id_tensor else None
    in_names, out_names, out_avals = [], [], []
    for alloc in nc.m.functions[0].allocations:
        if not isinstance(alloc, mybir.MemoryLocationSet):
            continue
        name = alloc.memorylocations[0].name
        if alloc.kind == "ExternalInput":
            if name != partition_name:
                in_names.append(name)
        elif alloc.kind == "ExternalOutput":
            out_names.append(name)
            out_avals.append(jax.core.ShapedArray(tuple(alloc.tensor_shape),
                                                  mybir.dt.np(alloc.dtype)))
    assert out_names == ["out"], out_names
    expect = [n for n in IN_NAMES if n != "out"]
    assert in_names == expect, (in_names, expect)
    all_names = in_names + out_names + ([partition_name] if partition_name else [])
    n_params = len(in_names)

    def _body(*args):
        operands = list(args)
        if partition_name is not None:
            operands.append(partition_id_tensor())
        return tuple(_bass_exec_p.bind(
            *operands, out_avals=tuple(out_avals), in_names=tuple(all_names),
            out_names=tuple(out_names), lowering_input_output_aliases=(),
            sim_require_finite=True, sim_require_nnan=True, nc=nc))

    donate = tuple(range(n_params, n_params + len(out_names)))
    sharded = jax.jit(
        shard_map(_body, mesh=mesh,
                  in_specs=(PartitionSpec("core"),) * (n_params + len(out_names)),
                  out_specs=(PartitionSpec("core"),) * len(out_names), check_rep=False),
        donate_argnums=donate, keep_unused=True)

    arg_specs = [specs[n] for n in in_names] + [specs["out"]]
    compiled = sharded.lower(*arg_specs).compile()
    t3 = time.perf_counter()
    print(f"[kernel] jit compile: {t3 - t2:.2f}s", file=sys.stderr, flush=True)

    up_th.join()
    t4 = time.perf_counter()
    print(f"[kernel] upload join: {t4 - t3:.2f}s", file=sys.stderr, flush=True)

    call_args = [dev[n] for n in in_names] + [dev["out"]]
    out_arrs = compiled(*call_args)
    jax.block_until_ready(out_arrs)
    t5 = time.perf_counter()
    print(f"[kernel] exec: {t5 - t4:.2f}s", file=sys.stderr, flush=True)
    kernel.last_exec_ns = None

    full = np.asarray(out_arrs[0]).reshape(n_cores, SHARD_P, D)
    out = np.zeros((N, D), np.float32)
    for c in range(n_cores):
        out[c * SHARD:(c + 1) * SHARD] = full[c][:SHARD]
    print(f"[kernel] post: {time.perf_counter() - t5:.2f}s", file=sys.stderr, flush=True)
    return out
